# revision 3
# baseline (speedup 1.0000x reference)
"""Trainium2 Bass kernel for per-batch (block-diagonal) attention.

Computes, for each batch b independently:
    q = x[b] @ Wq ; k = kv[b] @ Wk ; v = kv[b] @ Wv
    out[b] = softmax(q @ k^T) @ v

Sharding: data-parallel over B=8 across the 8 NeuronCores (one batch
element per core). Each core holds the full 64x64 weights.

Math used on-device (per core, x:[8192,64], kv:[1024,64]):
    A   = Wq @ Wk^T            (64x64, fp32)
    U^T = A  @ kv^T            (64x1024, fp32 -> fp16)
    S^T = U  @ x^T             -> scores^T tiles [128k, 512q] (fp16 in, fp32 acc)
    P^T = exp(S^T)             (ACT, PSUM->SBUF, bf16 out)
    outT_aug = [v | 1 | 0pad]^T @ P^T  (bf16, PSUM fp32 accumulate;
                                        row 64 = softmax denominator)
    out = outT_aug[0:64].T / denom   (PE transpose back + DVE reciprocal*mul)

The kernel is ACT(exp)-bound in steady state: 8.39M exps/core at
~128 lanes * 1.2 GHz ~= 55 us floor, +352cyc/ACTIVATE overhead.
Prologue is latency-optimized to start the first ACTIVATE ASAP:
 - DMA submits split across the two hwdge queues (sync + scalar),
   x chunk0 halves in parallel, kv kt0/kt1 first.
 - kv^T / U^T computed in fp32 piecewise (no f16 pre-cast of kv),
   so scores for key-tile 0 can start before the rest of kv lands.
"""

from contextlib import ExitStack

import numpy as np

import concourse.mybir as mybir
from concourse import bacc
from concourse.masks import make_identity
from concourse.tile import TileContext

B, LQ, LK, NF = 8, 8192, 1024, 64
P = 128
CH = 512             # queries per PSUM-bank-sized slice
KT = LK // P         # 8 key tiles
F32 = mybir.dt.float32
F16 = mybir.dt.float16
BF16 = mybir.dt.bfloat16
EXP = mybir.ActivationFunctionType.Exp

_CACHE: dict = {}


def _build_nc():
    nc = bacc.Bacc("TRN2", target_bir_lowering=False, debug=False)
    x = nc.dram_tensor("x", [LQ, NF], F32, kind="ExternalInput").ap()
    kv = nc.dram_tensor("kv", [LK, NF], F32, kind="ExternalInput").ap()
    wq = nc.dram_tensor("Wq", [NF, NF], F32, kind="ExternalInput").ap()
    wk = nc.dram_tensor("Wk", [NF, NF], F32, kind="ExternalInput").ap()
    wv = nc.dram_tensor("Wv", [NF, NF], F32, kind="ExternalInput").ap()
    y = nc.dram_tensor("y", [LQ, NF], F32, kind="ExternalOutput").ap()

    CP = 2 * CH  # 1024 queries per chunk-pair

    with TileContext(nc) as tc, ExitStack() as ctx:
        singles = ctx.enter_context(tc.tile_pool(name="singles", bufs=1))
        xin = ctx.enter_context(tc.tile_pool(name="xin", bufs=4))

        # preload the exp table set ASAP so the ~2.7us load overlaps prologue
        warm = singles.tile([P, 1], F32)
        nc.vector.memset(warm, 0.0)
        nc.scalar.activation(out=warm, in_=warm, func=EXP)

        # ---- DMA submits first, split across the sync + scalar queues ----
        # sync:   x c0 half A, kv kt2-7, then steady x prefetches
        # scalar: x c0 half B, wq, wk, kv kt0-1, wv  (scalar is free pre-exp)
        x_sb0 = xin.tile([P, 4, 2, NF], F32)
        xv0 = x[0:CP, :].rearrange("(i par p) f -> p i par f", p=P, par=2)
        kv_sb = singles.tile([P, KT, NF], F32)
        kv_v = kv.rearrange("(t p) f -> p t f", p=P)
        wq_sb = singles.tile([NF, NF], F32)
        wk_sb = singles.tile([NF, NF], F32)
        wv_sb = singles.tile([NF, NF], F32)

        nc.sync.dma_start(out=x_sb0[:, :2], in_=xv0[:, :2])
        nc.scalar.dma_start(out=x_sb0[:, 2:], in_=xv0[:, 2:])
        nc.scalar.dma_start(out=wq_sb, in_=wq)
        nc.scalar.dma_start(out=wk_sb, in_=wk)
        nc.sync.dma_start(out=kv_sb[:, 2:, :], in_=kv_v[:, 2:, :])
        nc.scalar.dma_start(out=kv_sb[:, :2, :], in_=kv_v[:, :2, :])
        nc.scalar.dma_start(out=wv_sb, in_=wv)

        ident = singles.tile([P, P], F32)
        make_identity(nc, ident)
        ident16 = singles.tile([P, P], F16)
        nc.gpsimd.tensor_copy(ident16, ident)
        identb = singles.tile([P, P], BF16)
        nc.gpsimd.tensor_copy(identb, ident)

        # ---- prologue compute: W^T, A^T, kv^T, U^T (all fp32) ----
        with tc.tile_pool(name="pro_ps", bufs=2, space="PSUM") as pro_ps:
            # W^T via PE transpose
            wqT = singles.tile([NF, NF], F32)
            wkT = singles.tile([NF, NF], F32)
            for w_sb, wT in ((wq_sb, wqT), (wk_sb, wkT)):
                t_ps = pro_ps.tile([NF, NF], F32, tag="pro")
                nc.tensor.transpose(t_ps, w_sb, ident[:NF, :NF])
                nc.vector.tensor_copy(wT, t_ps)

            # A^T = Wk @ Wq^T  (= (Wq Wk^T)^T), fp32
            at_ps = pro_ps.tile([NF, NF], F32, tag="pro")
            nc.tensor.matmul(at_ps, lhsT=wkT, rhs=wqT, start=True, stop=True)
            aT32 = singles.tile([NF, NF], F32)
            nc.vector.tensor_copy(aT32, at_ps)
            wv16 = singles.tile([NF, NF], F16)
            nc.gpsimd.tensor_copy(wv16, wv_sb)

            # kv^T [64, 1024] fp32, piecewise: kt0, kt1 solo (fast path),
            # then stacked pairs for kt2-7
            kvT32 = singles.tile([NF, LK], F32)
            kvT16 = singles.tile([NF, LK], F16)
            for t in range(2):
                kt_ps = pro_ps.tile([NF, P], F32, tag="prokv")
                nc.tensor.transpose(kt_ps, kv_sb[:, t, :], ident)
                nc.vector.tensor_copy(kvT32[:, t * P : (t + 1) * P], kt_ps)
            for i in range(1, KT // 2):
                kt2_ps = pro_ps.tile([P, P], F32, tag="prokv2")
                nc.tensor.transpose(kt2_ps, kv_sb[:, 2 * i : 2 * i + 2, :], ident)
                nc.vector.tensor_copy(
                    kvT32[:, (2 * i) * P : (2 * i + 1) * P], kt2_ps[:NF, :]
                )
                nc.vector.tensor_copy(
                    kvT32[:, (2 * i + 1) * P : (2 * i + 2) * P], kt2_ps[NF:, :]
                )

            # U^T = A @ kv^T, fp32 matmul piecewise, cast to fp16 on drain;
            # duplicated into partitions 64:128 for the packed row-group MMs
            uT = singles.tile([P, LK], F16)
            for c0, c1 in ((0, 256), (256, 512), (512, 768), (768, 1024)):
                ut_ps = pro_ps.tile([NF, 256], F32, tag="prou")
                nc.tensor.matmul(
                    ut_ps, lhsT=aT32, rhs=kvT32[:, c0:c1], start=True, stop=True
                )
                nc.vector.tensor_copy(uT[:NF, c0:c1], ut_ps)
                nc.vector.tensor_copy(uT[NF:, c0:c1], uT[:NF, c0:c1])
                nc.vector.tensor_copy(kvT16[:, c0:c1], kvT32[:, c0:c1])

            # v_aug is filled later (inside chunk-pair 0, after its scores are
            # queued) so the first exp doesn't wait behind the v matmuls
            v_aug = singles.tile([P, KT, P], BF16)
            ones_sb = singles.tile([P, 1], F32)
            nc.vector.memset(ones_sb, 1.0)

        # ---- main loop over query chunk-pairs (1024 queries each) ----
        x16_pool = ctx.enter_context(tc.tile_pool(name="x16", bufs=4))
        xT_pool = ctx.enter_context(tc.tile_pool(name="xT", bufs=3))
        pT_pool = ctx.enter_context(tc.tile_pool(name="pT", bufs=12))
        pvT_pool = ctx.enter_context(tc.tile_pool(name="pvT", bufs=3))
        out_pool = ctx.enter_context(tc.tile_pool(name="outsb", bufs=3))
        rec_pool = ctx.enter_context(tc.tile_pool(name="rec", bufs=4))

        xt_ps_pool = ctx.enter_context(
            tc.tile_pool(name="xt_ps", bufs=1, space="PSUM")
        )
        ot_ps_pool = ctx.enter_context(
            tc.tile_pool(name="ot_ps", bufs=1, space="PSUM")
        )
        sc_ps_pool = ctx.enter_context(
            tc.tile_pool(name="sc_ps", bufs=2, space="PSUM")
        )
        pv_ps_pool = ctx.enter_context(
            tc.tile_pool(name="pv_ps", bufs=1, space="PSUM")
        )

        for c in range(LQ // CP):
            # one DMA; subtile pairs side by side for stacked PE transposes
            if c == 0:
                x_sb = x_sb0
            else:
                x_sb = xin.tile([P, 4, 2, NF], F32)
                nc.sync.dma_start(
                    out=x_sb,
                    in_=x[c * CP : (c + 1) * CP, :].rearrange(
                        "(i par p) f -> p i par f", p=P, par=2
                    ),
                )
            # cast to fp16 on the (otherwise idle) gpsimd engine, then
            # stacked transpose: xt partitions 0:64 = even subtiles' features,
            # 64:128 = odd subtiles' features
            x16 = x16_pool.tile([P, 4, 2, NF], F16)
            if c == 0:
                # split across gpsimd + vector to cut chunk-0 latency
                nc.gpsimd.tensor_copy(x16[:, :2], x_sb[:, :2])
                nc.vector.tensor_copy(x16[:, 2:], x_sb[:, 2:])
            else:
                nc.gpsimd.tensor_copy(x16, x_sb)
            xt_ps = xt_ps_pool.tile([P, 4, P], F16, tag="xt")
            for i in range(4):
                nc.tensor.transpose(xt_ps[:, i, :], x16[:, i, :, :], ident16)
            xTc = xT_pool.tile([P, 4, P], F16)
            nc.vector.tensor_copy(xTc, xt_ps)

            # scores^T: per key tile, 2 row-group-packed MMs (even/odd qs)
            # -> exp -> P^T [128, KT, 1024] (bf16)
            pTs = []
            for t in range(KT):
                s_ps = sc_ps_pool.tile([P, CP], F32)
                nc.tensor.matmul(
                    s_ps[:, :CH],
                    lhsT=uT[:NF, t * P : (t + 1) * P],
                    rhs=xTc[:NF],
                    start=True, stop=True,
                    tile_position=(0, 0),
                )
                nc.tensor.matmul(
                    s_ps[:, CH:],
                    lhsT=uT[NF:, t * P : (t + 1) * P],
                    rhs=xTc[NF:],
                    start=True, stop=True,
                    tile_position=(64, 0),
                )
                pT_t = pT_pool.tile([P, CP], BF16, tag="pT")
                pTs.append(pT_t)
                nc.scalar.activation(out=pT_t, in_=s_ps, func=EXP)

            if c == 0:
                # fill v_aug now: [v | 1 | 0pad] per key tile, bf16
                for t in range(KT):
                    v_ps = xt_ps_pool.tile([P, NF], F32, tag="xt")
                    nc.tensor.matmul(
                        v_ps, lhsT=kvT16[:, t * P : (t + 1) * P], rhs=wv16,
                        start=True, stop=True,
                    )
                    nc.vector.tensor_copy(v_aug[:, t, :NF], v_ps)
                    nc.vector.tensor_copy(v_aug[:, t, NF : NF + 1], ones_sb)
                    nc.vector.memset(v_aug[:, t, NF + 1 :], 0.0)

            # PV: outT_aug [128, 1024] accumulated over key tiles (row 64 = denom)
            pv_ps = pv_ps_pool.tile([P, CP], F32)
            for t in range(KT):
                for half in range(2):
                    nc.tensor.matmul(
                        pv_ps[:, half * CH : (half + 1) * CH],
                        lhsT=v_aug[:, t, :],
                        rhs=pTs[t][:, half * CH : (half + 1) * CH],
                        start=(t == 0), stop=(t == KT - 1),
                    )
            pvT = pvT_pool.tile([NF + 1, CP], BF16)
            nc.vector.tensor_copy(pvT[:, :CH], pv_ps[: NF + 1, :CH])
            nc.vector.tensor_copy(pvT[:, CH:], pv_ps[: NF + 1, CH:])

            # transpose back to [128 q, 65], normalize, store
            out_sb = out_pool.tile([P, 4, 2, NF], F32)
            for r in range(2):
                ot_ps = ot_ps_pool.tile([P, 4, NF + 2], BF16)
                rec = rec_pool.tile([P, 4], F32)
                for s in range(4):
                    j = 4 * r + s
                    nc.tensor.transpose(
                        ot_ps[:, s, : NF + 1],
                        pvT[:, j * P : (j + 1) * P],
                        identb[: NF + 1, : NF + 1],
                    )
                nc.vector.reciprocal(rec, ot_ps[:, :, NF])
                for s in range(4):
                    nc.vector.tensor_scalar_mul(
                        out_sb[:, s, r, :], ot_ps[:, s, :NF], rec[:, s : s + 1]
                    )
            # column j of pvT maps to q = c*1024 + (2*i + r)*128 + b where
            # j = r*512 + i*128 + b  =>  y viewed as (s r p) with s=i
            y_v = y[c * CP : (c + 1) * CP, :].rearrange(
                "(s r p) f -> p s r f", p=P, r=2
            )
            for r in range(2):
                nc.sync.dma_start(out=y_v[:, :, r, :], in_=out_sb[:, :, r, :])

    nc.compile()
    return nc


def get_nc():
    if "nc" not in _CACHE:
        _CACHE["nc"] = _build_nc()
    return _CACHE["nc"]


def run(inputs: dict, trace: bool = False):
    """Run on the 8 NeuronCores. Returns (out [8,8192,64], exec_time_ns)."""
    from concourse.bass_utils import run_bass_kernel_spmd

    nc = get_nc()
    in_maps = [
        {
            "x": np.ascontiguousarray(inputs["x"][b]),
            "kv": np.ascontiguousarray(inputs["kv"][b]),
            "Wq": np.asarray(inputs["Wq"]),
            "Wk": np.asarray(inputs["Wk"]),
            "Wv": np.asarray(inputs["Wv"]),
        }
        for b in range(B)
    ]
    res = run_bass_kernel_spmd(
        nc, in_maps, core_ids=list(range(B)), trace=trace
    )
    out = np.stack([res.results[b]["y"] for b in range(B)])
    return out, res.exec_time_ns


def kernel(**inputs) -> np.ndarray:
    out, _ = run(inputs, trace=False)
    return out


# revision 5
# speedup vs baseline: 1.0533x; 1.0533x over previous
"""Trainium2 Bass kernel for per-batch (block-diagonal) attention.

Computes, for each batch b independently:
    q = x[b] @ Wq ; k = kv[b] @ Wk ; v = kv[b] @ Wv
    out[b] = softmax(q @ k^T) @ v

Sharding: data-parallel over B=8 across the 8 NeuronCores (one batch
element per core). Each core holds the full 64x64 weights.

Math used on-device (per core, x:[8192,64], kv:[1024,64]):
    A   = Wq @ Wk^T            (64x64, fp32)
    U^T = A  @ kv^T            (64x1024, f16 MM from f16 kv^T)
    S^T = U  @ x^T             -> scores^T tiles [128k, 512q] (fp16 in, fp32 acc)
    P^T = exp(S^T)             (ACT, PSUM->SBUF, bf16 out)
    outT_aug = [v | 1 | 0pad]^T @ P^T  (bf16, PSUM fp32 accumulate;
                                        row 64 = softmax denominator)
    out = outT_aug[0:64].T / denom   (PE transpose back + DVE reciprocal*mul)

The kernel is ACT(exp)-bound in steady state: 8.39M exps/core at
128 lanes * 1.2 GHz ~= 55 us floor, +~300cyc/ACTIVATE overhead.
Prologue is latency-optimized for the first ACTIVATE:
 - DMA submits split across the two hwdge queues: sync gets x chunk0
   (two halves) then kv kt2-7; scalar gets wq, wk, kv kt0-1, wv.
 - kv kt0-1 are cast/transposed/U^T'd as a separate piece so scores for
   key-tile 0 start before the rest of kv is even on-chip.
 - casts balanced across gpsimd (x half A, kv) and vector (x half B).
"""

from contextlib import ExitStack

import numpy as np

import concourse.mybir as mybir
from concourse import bacc
from concourse.masks import make_identity
from concourse.tile import TileContext

B, LQ, LK, NF = 8, 8192, 1024, 64
P = 128
CH = 512             # queries per PSUM-bank-sized slice
KT = LK // P         # 8 key tiles
F32 = mybir.dt.float32
F16 = mybir.dt.float16
BF16 = mybir.dt.bfloat16
EXP = mybir.ActivationFunctionType.Exp

_CACHE: dict = {}


def _build_nc():
    nc = bacc.Bacc("TRN2", target_bir_lowering=False, debug=False)
    x = nc.dram_tensor("x", [LQ, NF], F32, kind="ExternalInput").ap()
    kv = nc.dram_tensor("kv", [LK, NF], F32, kind="ExternalInput").ap()
    wq = nc.dram_tensor("Wq", [NF, NF], F32, kind="ExternalInput").ap()
    wk = nc.dram_tensor("Wk", [NF, NF], F32, kind="ExternalInput").ap()
    wv = nc.dram_tensor("Wv", [NF, NF], F32, kind="ExternalInput").ap()
    y = nc.dram_tensor("y", [LQ, NF], F32, kind="ExternalOutput").ap()

    CP = 2 * CH  # 1024 queries per chunk-pair

    with TileContext(nc) as tc, ExitStack() as ctx:
        singles = ctx.enter_context(tc.tile_pool(name="singles", bufs=1))
        xin = ctx.enter_context(tc.tile_pool(name="xin", bufs=4))

        # preload the exp table set ASAP so the ~2.7us load overlaps prologue
        warm = singles.tile([P, 1], F32)
        nc.vector.memset(warm, 0.0)
        nc.scalar.activation(out=warm, in_=warm, func=EXP)

        # ---- DMA submits first, split across the sync + scalar queues ----
        x_sb0 = xin.tile([P, 4, 2, NF], F32)
        xv0 = x[0:CP, :].rearrange("(i par p) f -> p i par f", p=P, par=2)
        kv_sb = singles.tile([P, KT, NF], F32)
        kv_v = kv.rearrange("(t p) f -> p t f", p=P)
        wq_sb = singles.tile([NF, NF], F32)
        wk_sb = singles.tile([NF, NF], F32)
        wv_sb = singles.tile([NF, NF], F32)

        nc.sync.dma_start(out=x_sb0[:, :2], in_=xv0[:, :2])
        nc.sync.dma_start(out=x_sb0[:, 2:], in_=xv0[:, 2:])
        nc.scalar.dma_start(out=wq_sb, in_=wq)
        nc.scalar.dma_start(out=wk_sb, in_=wk)
        nc.scalar.dma_start(out=kv_sb[:, :2, :], in_=kv_v[:, :2, :])
        nc.sync.dma_start(out=kv_sb[:, 2:, :], in_=kv_v[:, 2:, :])
        nc.scalar.dma_start(out=wv_sb, in_=wv)

        ident = singles.tile([P, P], F32)
        make_identity(nc, ident)
        ident16 = singles.tile([P, P], F16)
        nc.gpsimd.tensor_copy(ident16, ident)
        identb = singles.tile([P, P], BF16)
        nc.gpsimd.tensor_copy(identb, ident)

        # ---- prologue compute: W^T, A^T(f16), kv^T(f16), U^T(f16) ----
        with tc.tile_pool(name="pro_ps", bufs=2, space="PSUM") as pro_ps:
            # W^T via PE transpose (fp32), A^T = Wk @ Wq^T in fp32, cast f16
            wqT = singles.tile([NF, NF], F32)
            wkT = singles.tile([NF, NF], F32)
            for w_sb, wT in ((wq_sb, wqT), (wk_sb, wkT)):
                t_ps = pro_ps.tile([NF, NF], F32, tag="pro")
                nc.tensor.transpose(t_ps, w_sb, ident[:NF, :NF])
                nc.vector.tensor_copy(wT, t_ps)
            at_ps = pro_ps.tile([NF, NF], F32, tag="pro")
            nc.tensor.matmul(at_ps, lhsT=wkT, rhs=wqT, start=True, stop=True)
            aT16 = singles.tile([NF, NF], F16)
            nc.vector.tensor_copy(aT16, at_ps)
            wv16 = singles.tile([NF, NF], F16)
            nc.gpsimd.tensor_copy(wv16, wv_sb)

            # kv -> f16 -> kv^T [64, 1024] f16, piecewise: kt0-1 first
            kv16 = singles.tile([P, KT, NF], F16)
            kvT16 = singles.tile([NF, LK], F16)
            uT = singles.tile([P, LK], F16)

            def kv_piece(i0, i1):
                # cast + stacked transposes + kvT copies for kt pairs i0..i1
                nc.gpsimd.tensor_copy(
                    kv16[:, 2 * i0 : 2 * i1, :], kv_sb[:, 2 * i0 : 2 * i1, :]
                )
                for i in range(i0, i1):
                    kt_ps = pro_ps.tile([P, P], F16, tag="prokv")
                    nc.tensor.transpose(
                        kt_ps, kv16[:, 2 * i : 2 * i + 2, :], ident16
                    )
                    nc.vector.tensor_copy(
                        kvT16[:, (2 * i) * P : (2 * i + 1) * P], kt_ps[:NF, :]
                    )
                    nc.vector.tensor_copy(
                        kvT16[:, (2 * i + 1) * P : (2 * i + 2) * P], kt_ps[NF:, :]
                    )
                # U^T piece: f16 MM, cast to f16 with row-dup for packed MMs
                c0, c1 = (2 * i0) * P, (2 * i1) * P
                for d0 in range(c0, c1, CH):
                    d1 = min(d0 + CH, c1)
                    ut_ps = pro_ps.tile([NF, CH], F32, tag="prou")
                    nc.tensor.matmul(
                        ut_ps[:, : d1 - d0], lhsT=aT16, rhs=kvT16[:, d0:d1],
                        start=True, stop=True,
                    )
                    nc.vector.tensor_copy(uT[:NF, d0:d1], ut_ps[:, : d1 - d0])
                    nc.vector.tensor_copy(uT[NF:, d0:d1], uT[:NF, d0:d1])

            kv_piece(0, 1)   # kt0-1: fast path for the first score panels
            kv_piece(1, 2)   # kt2-3
            kv_piece(2, 4)   # kt4-7

            # v_aug is filled later (inside chunk-pair 0, after its scores are
            # queued) so the first exp doesn't wait behind the v matmuls
            v_aug = singles.tile([P, KT, P], BF16)
            ones_sb = singles.tile([P, 1], F32)
            nc.vector.memset(ones_sb, 1.0)

        # ---- main loop over query chunk-pairs (1024 queries each) ----
        x16_pool = ctx.enter_context(tc.tile_pool(name="x16", bufs=4))
        xT_pool = ctx.enter_context(tc.tile_pool(name="xT", bufs=3))
        pT_pool = ctx.enter_context(tc.tile_pool(name="pT", bufs=12))
        pvT_pool = ctx.enter_context(tc.tile_pool(name="pvT", bufs=3))
        out_pool = ctx.enter_context(tc.tile_pool(name="outsb", bufs=3))
        rec_pool = ctx.enter_context(tc.tile_pool(name="rec", bufs=4))

        xt_ps_pool = ctx.enter_context(
            tc.tile_pool(name="xt_ps", bufs=1, space="PSUM")
        )
        ot_ps_pool = ctx.enter_context(
            tc.tile_pool(name="ot_ps", bufs=1, space="PSUM")
        )
        sc_ps_pool = ctx.enter_context(
            tc.tile_pool(name="sc_ps", bufs=2, space="PSUM")
        )
        pv_ps_pool = ctx.enter_context(
            tc.tile_pool(name="pv_ps", bufs=1, space="PSUM")
        )

        for c in range(LQ // CP):
            # one DMA; subtile pairs side by side for stacked PE transposes
            if c == 0:
                x_sb = x_sb0
            else:
                x_sb = xin.tile([P, 4, 2, NF], F32)
                nc.sync.dma_start(
                    out=x_sb,
                    in_=x[c * CP : (c + 1) * CP, :].rearrange(
                        "(i par p) f -> p i par f", p=P, par=2
                    ),
                )
            # cast to fp16 on the (otherwise idle) gpsimd engine, then
            # stacked transpose: xt partitions 0:64 = even subtiles' features,
            # 64:128 = odd subtiles' features
            x16 = x16_pool.tile([P, 4, 2, NF], F16)
            if c == 0:
                # split across gpsimd + vector to cut chunk-0 latency
                nc.gpsimd.tensor_copy(x16[:, :2], x_sb[:, :2])
                nc.vector.tensor_copy(x16[:, 2:], x_sb[:, 2:])
            else:
                nc.gpsimd.tensor_copy(x16, x_sb)
            xt_ps = xt_ps_pool.tile([P, 4, P], F16, tag="xt")
            for i in range(4):
                nc.tensor.transpose(xt_ps[:, i, :], x16[:, i, :, :], ident16)
            xTc = xT_pool.tile([P, 4, P], F16)
            nc.vector.tensor_copy(xTc, xt_ps)

            # scores^T: per key tile, 2 row-group-packed MMs (even/odd qs)
            # -> exp -> P^T [128, KT, 1024] (bf16)
            pTs = []
            for t in range(KT):
                s_ps = sc_ps_pool.tile([P, CP], F32)
                nc.tensor.matmul(
                    s_ps[:, :CH],
                    lhsT=uT[:NF, t * P : (t + 1) * P],
                    rhs=xTc[:NF],
                    start=True, stop=True,
                    tile_position=(0, 0),
                )
                nc.tensor.matmul(
                    s_ps[:, CH:],
                    lhsT=uT[NF:, t * P : (t + 1) * P],
                    rhs=xTc[NF:],
                    start=True, stop=True,
                    tile_position=(64, 0),
                )
                pT_t = pT_pool.tile([P, CP], BF16, tag="pT")
                pTs.append(pT_t)
                nc.scalar.activation(out=pT_t, in_=s_ps, func=EXP)

            if c == 0:
                # fill v_aug now: [v | 1 | 0pad] per key tile, bf16
                for t in range(KT):
                    v_ps = xt_ps_pool.tile([P, NF], F32, tag="xt")
                    nc.tensor.matmul(
                        v_ps, lhsT=kvT16[:, t * P : (t + 1) * P], rhs=wv16,
                        start=True, stop=True,
                    )
                    nc.vector.tensor_copy(v_aug[:, t, :NF], v_ps)
                    nc.vector.tensor_copy(v_aug[:, t, NF : NF + 1], ones_sb)
                    nc.vector.memset(v_aug[:, t, NF + 1 :], 0.0)

            # PV: outT_aug [128, 1024] accumulated over key tiles (row 64 = denom)
            pv_ps = pv_ps_pool.tile([P, CP], F32)
            for t in range(KT):
                for half in range(2):
                    nc.tensor.matmul(
                        pv_ps[:, half * CH : (half + 1) * CH],
                        lhsT=v_aug[:, t, :],
                        rhs=pTs[t][:, half * CH : (half + 1) * CH],
                        start=(t == 0), stop=(t == KT - 1),
                    )
            pvT = pvT_pool.tile([NF + 1, CP], BF16)
            nc.vector.tensor_copy(pvT[:, :CH], pv_ps[: NF + 1, :CH])
            nc.vector.tensor_copy(pvT[:, CH:], pv_ps[: NF + 1, CH:])

            # transpose back to [128 q, 65], normalize, store
            out_sb = out_pool.tile([P, 4, 2, NF], F32)
            for r in range(2):
                ot_ps = ot_ps_pool.tile([P, 4, NF + 2], BF16)
                rec = rec_pool.tile([P, 4], F32)
                for s in range(4):
                    j = 4 * r + s
                    nc.tensor.transpose(
                        ot_ps[:, s, : NF + 1],
                        pvT[:, j * P : (j + 1) * P],
                        identb[: NF + 1, : NF + 1],
                    )
                nc.vector.reciprocal(rec, ot_ps[:, :, NF])
                # single batched normalize: out = ot * rec (broadcast over f)
                nc.vector.tensor_tensor(
                    out_sb[:, :, r, :],
                    ot_ps[:, :, :NF],
                    rec.unsqueeze(2).broadcast_to([P, 4, NF]),
                    mybir.AluOpType.mult,
                )
            # column j of pvT maps to q = c*1024 + (2*i + r)*128 + b where
            # j = r*512 + i*128 + b  =>  y viewed as (s r p) with s=i
            y_v = y[c * CP : (c + 1) * CP, :].rearrange(
                "(s r p) f -> p s r f", p=P, r=2
            )
            for r in range(2):
                nc.sync.dma_start(out=y_v[:, :, r, :], in_=out_sb[:, :, r, :])

    nc.compile()
    return nc


def get_nc():
    if "nc" not in _CACHE:
        _CACHE["nc"] = _build_nc()
    return _CACHE["nc"]


def run(inputs: dict, trace: bool = False):
    """Run on the 8 NeuronCores. Returns (out [8,8192,64], exec_time_ns)."""
    from concourse.bass_utils import run_bass_kernel_spmd

    nc = get_nc()
    in_maps = [
        {
            "x": np.ascontiguousarray(inputs["x"][b]),
            "kv": np.ascontiguousarray(inputs["kv"][b]),
            "Wq": np.asarray(inputs["Wq"]),
            "Wk": np.asarray(inputs["Wk"]),
            "Wv": np.asarray(inputs["Wv"]),
        }
        for b in range(B)
    ]
    res = run_bass_kernel_spmd(
        nc, in_maps, core_ids=list(range(B)), trace=trace
    )
    out = np.stack([res.results[b]["y"] for b in range(B)])
    return out, res.exec_time_ns


def kernel(**inputs) -> np.ndarray:
    out, _ = run(inputs, trace=False)
    return out


# revision 6
# speedup vs baseline: 1.0709x; 1.0167x over previous
"""Trainium2 Bass kernel for per-batch (block-diagonal) attention.

Computes, for each batch b independently:
    q = x[b] @ Wq ; k = kv[b] @ Wk ; v = kv[b] @ Wv
    out[b] = softmax(q @ k^T) @ v

Sharding: data-parallel over B=8 across the 8 NeuronCores (one batch
element per core). Each core holds the full 64x64 weights.

Math used on-device (per core, x:[8192,64], kv:[1024,64]):
    A   = Wq @ Wk^T            (64x64, fp32)
    U^T = A  @ kv^T            (64x1024, f16)
    S^T = U  @ x^T             -> scores^T tiles [128k, 512q] (fp16 in, fp32 acc)
    P^T = exp(S^T)             (ACT, PSUM->SBUF, bf16 out)
    outT_aug = [v | 1 | 0pad]^T @ P^T  (bf16, PSUM fp32 accumulate;
                                        row 64 = softmax denominator)
    out = outT_aug[0:64].T / denom   (PE transpose back + DVE recip * mul)

Layout trick: SBUF tiles hold 8 *consecutive* HBM rows per partition
(row n = 8p + j), so every big DMA (x in, kv in, y out) moves 2KB
contiguous per partition (128 descriptors instead of 512) - ~4x faster
transfers and lower latency.  This permutes key order (tile t = keys
congruent t mod 8) and query order (q = 8p + 2i + half); key order
cancels identically through U^T/P^T/v_aug, and the y store view
inverts the query permutation.

The kernel is ACT(exp)-bound in steady state: 8.39M exps/core at
128 lanes * 1.2 GHz ~= 55 us floor, +~300cyc/ACTIVATE overhead.
"""

from contextlib import ExitStack

import numpy as np

import concourse.mybir as mybir
from concourse import bacc
from concourse.masks import make_identity
from concourse.tile import TileContext

B, LQ, LK, NF = 8, 8192, 1024, 64
P = 128
CH = 512             # queries per PSUM-bank-sized slice
KT = LK // P         # 8 key tiles
F32 = mybir.dt.float32
F16 = mybir.dt.float16
BF16 = mybir.dt.bfloat16
EXP = mybir.ActivationFunctionType.Exp

_CACHE: dict = {}


def _build_nc():
    nc = bacc.Bacc("TRN2", target_bir_lowering=False, debug=False)
    x = nc.dram_tensor("x", [LQ, NF], F32, kind="ExternalInput").ap()
    kv = nc.dram_tensor("kv", [LK, NF], F32, kind="ExternalInput").ap()
    wq = nc.dram_tensor("Wq", [NF, NF], F32, kind="ExternalInput").ap()
    wk = nc.dram_tensor("Wk", [NF, NF], F32, kind="ExternalInput").ap()
    wv = nc.dram_tensor("Wv", [NF, NF], F32, kind="ExternalInput").ap()
    y = nc.dram_tensor("y", [LQ, NF], F32, kind="ExternalOutput").ap()

    CP = 2 * CH  # 1024 queries per chunk-pair

    with TileContext(nc) as tc, ExitStack() as ctx:
        singles = ctx.enter_context(tc.tile_pool(name="singles", bufs=1))
        xin = ctx.enter_context(tc.tile_pool(name="xin", bufs=4))

        # preload the exp table set ASAP so the ~2.7us load overlaps prologue
        warm = singles.tile([P, 1], F32)
        nc.vector.memset(warm, 0.0)
        nc.scalar.activation(out=warm, in_=warm, func=EXP)

        # ---- DMA submits first, split across the sync + scalar queues ----
        # All big transfers use the 8-consecutive-rows-per-partition layout
        # (2KB contiguous descriptors).
        x_sb0 = xin.tile([P, 8, NF], F32)
        kv_sb = singles.tile([P, 8, NF], F32)
        wq_sb = singles.tile([NF, NF], F32)
        wk_sb = singles.tile([NF, NF], F32)
        wv_sb = singles.tile([NF, NF], F32)

        nc.sync.dma_start(
            out=x_sb0, in_=x[0:CP, :].rearrange("(p j) f -> p j f", p=P)
        )
        nc.scalar.dma_start(out=wq_sb, in_=wq)
        nc.sync.dma_start(out=wk_sb, in_=wk)
        nc.scalar.dma_start(
            out=kv_sb, in_=kv.rearrange("(p r) f -> p r f", p=P)
        )
        nc.scalar.dma_start(out=wv_sb, in_=wv)

        ident = singles.tile([P, P], F32)
        make_identity(nc, ident)
        ident16 = singles.tile([P, P], F16)
        nc.gpsimd.tensor_copy(ident16, ident)
        identb = singles.tile([P, P], BF16)
        nc.gpsimd.tensor_copy(identb, ident)

        # ---- prologue compute: W^T, A^T(f16), kv^T(f16), U^T(f16) ----
        with tc.tile_pool(name="pro_ps", bufs=2, space="PSUM") as pro_ps:
            # W^T via PE transpose (fp32), A^T = Wk @ Wq^T in fp32, cast f16
            wqT = singles.tile([NF, NF], F32)
            wkT = singles.tile([NF, NF], F32)
            for w_sb, wT in ((wq_sb, wqT), (wk_sb, wkT)):
                t_ps = pro_ps.tile([NF, NF], F32, tag="pro")
                nc.tensor.transpose(t_ps, w_sb, ident[:NF, :NF])
                nc.vector.tensor_copy(wT, t_ps)
            at_ps = pro_ps.tile([NF, NF], F32, tag="pro")
            nc.tensor.matmul(at_ps, lhsT=wkT, rhs=wqT, start=True, stop=True)
            aT16 = singles.tile([NF, NF], F16)
            nc.vector.tensor_copy(aT16, at_ps)
            wv16 = singles.tile([NF, NF], F16)
            nc.gpsimd.tensor_copy(wv16, wv_sb)

            # kv -> f16 -> kv^T [64, 1024] f16, piecewise: kt0-1 first.
            # key-tile t holds keys {8j + t}; order is consistent across
            # U^T, P^T and v_aug so the softmax result is unchanged.
            kv16 = singles.tile([P, 8, NF], F16)
            kvT16 = singles.tile([NF, LK], F16)
            uT = singles.tile([P, LK], F16)

            def kv_piece(i0, i1):
                nc.gpsimd.tensor_copy(
                    kv16[:, 2 * i0 : 2 * i1, :], kv_sb[:, 2 * i0 : 2 * i1, :]
                )
                for i in range(i0, i1):
                    kt_ps = pro_ps.tile([P, P], F16, tag="prokv")
                    nc.tensor.transpose(
                        kt_ps, kv16[:, 2 * i : 2 * i + 2, :], ident16
                    )
                    nc.vector.tensor_copy(
                        kvT16[:, (2 * i) * P : (2 * i + 1) * P], kt_ps[:NF, :]
                    )
                    nc.vector.tensor_copy(
                        kvT16[:, (2 * i + 1) * P : (2 * i + 2) * P], kt_ps[NF:, :]
                    )
                # U^T piece: f16 MM; cast twice from PSUM (partitions 0:64
                # and the 64:128 duplicate for the packed row-group MMs)
                c0, c1 = (2 * i0) * P, (2 * i1) * P
                for d0 in range(c0, c1, CH):
                    d1 = min(d0 + CH, c1)
                    ut_ps = pro_ps.tile([NF, CH], F32, tag="prou")
                    nc.tensor.matmul(
                        ut_ps[:, : d1 - d0], lhsT=aT16, rhs=kvT16[:, d0:d1],
                        start=True, stop=True,
                    )
                    nc.vector.tensor_copy(uT[:NF, d0:d1], ut_ps[:, : d1 - d0])
                    nc.vector.tensor_copy(uT[NF:, d0:d1], ut_ps[:, : d1 - d0])

            kv_piece(0, 1)   # kt0-1: fast path for the first score panels
            kv_piece(1, 2)   # kt2-3
            kv_piece(2, 4)   # kt4-7

            # v_aug is filled later (inside chunk-pair 0, after its scores are
            # queued) so the first exp doesn't wait behind the v matmuls
            v_aug = singles.tile([P, KT, P], BF16)
            ones_sb = singles.tile([P, 1], F32)
            nc.vector.memset(ones_sb, 1.0)

        # ---- main loop over query chunk-pairs (1024 queries each) ----
        x16_pool = ctx.enter_context(tc.tile_pool(name="x16", bufs=4))
        xT_pool = ctx.enter_context(tc.tile_pool(name="xT", bufs=3))
        pT_pool = ctx.enter_context(tc.tile_pool(name="pT", bufs=12))
        pvT_pool = ctx.enter_context(tc.tile_pool(name="pvT", bufs=3))
        out_pool = ctx.enter_context(tc.tile_pool(name="outsb", bufs=3))
        rec_pool = ctx.enter_context(tc.tile_pool(name="rec", bufs=4))

        xt_ps_pool = ctx.enter_context(
            tc.tile_pool(name="xt_ps", bufs=1, space="PSUM")
        )
        ot_ps_pool = ctx.enter_context(
            tc.tile_pool(name="ot_ps", bufs=1, space="PSUM")
        )
        sc_ps_pool = ctx.enter_context(
            tc.tile_pool(name="sc_ps", bufs=2, space="PSUM")
        )
        pv_ps_pool = ctx.enter_context(
            tc.tile_pool(name="pv_ps", bufs=1, space="PSUM")
        )

        for c in range(LQ // CP):
            if c == 0:
                x_sb = x_sb0
            else:
                x_sb = xin.tile([P, 8, NF], F32)
                nc.sync.dma_start(
                    out=x_sb,
                    in_=x[c * CP : (c + 1) * CP, :].rearrange(
                        "(p j) f -> p j f", p=P
                    ),
                )
            # cast to fp16, then stacked transpose: xt partitions 0:64 =
            # even-j queries' features, 64:128 = odd-j queries'
            x16 = x16_pool.tile([P, 8, NF], F16)
            if c == 0:
                # split across vector halves to cut chunk-0 latency
                # (gpsimd is busy casting kv)
                nc.vector.tensor_copy(x16[:, :4], x_sb[:, :4])
                nc.vector.tensor_copy(x16[:, 4:], x_sb[:, 4:])
            else:
                nc.gpsimd.tensor_copy(x16, x_sb)
            xt_ps = xt_ps_pool.tile([P, 4, P], F16, tag="xt")
            for i in range(4):
                nc.tensor.transpose(
                    xt_ps[:, i, :], x16[:, 2 * i : 2 * i + 2, :], ident16
                )
            xTc = xT_pool.tile([P, 4, P], F16)
            nc.vector.tensor_copy(xTc, xt_ps)

            # scores^T: per key tile, 2 row-group-packed MMs (even/odd qs)
            # -> exp -> P^T [128, KT, 1024] (bf16)
            pTs = []
            for t in range(KT):
                s_ps = sc_ps_pool.tile([P, CP], F32)
                nc.tensor.matmul(
                    s_ps[:, :CH],
                    lhsT=uT[:NF, t * P : (t + 1) * P],
                    rhs=xTc[:NF],
                    start=True, stop=True,
                    tile_position=(0, 0),
                )
                nc.tensor.matmul(
                    s_ps[:, CH:],
                    lhsT=uT[NF:, t * P : (t + 1) * P],
                    rhs=xTc[NF:],
                    start=True, stop=True,
                    tile_position=(64, 0),
                )
                pT_t = pT_pool.tile([P, CP], BF16, tag="pT")
                pTs.append(pT_t)
                nc.scalar.activation(out=pT_t, in_=s_ps, func=EXP)

            if c == 0:
                # fill v_aug now: [v | 1 | 0pad] per key tile, bf16
                for t in range(KT):
                    v_ps = xt_ps_pool.tile([P, NF], F32, tag="xt")
                    nc.tensor.matmul(
                        v_ps, lhsT=kvT16[:, t * P : (t + 1) * P], rhs=wv16,
                        start=True, stop=True,
                    )
                    nc.vector.tensor_copy(v_aug[:, t, :NF], v_ps)
                    nc.vector.tensor_copy(v_aug[:, t, NF : NF + 1], ones_sb)
                    nc.vector.memset(v_aug[:, t, NF + 1 :], 0.0)

            # PV: outT_aug [128, 1024] accumulated over key tiles (row 64 = denom)
            pv_ps = pv_ps_pool.tile([P, CP], F32)
            for t in range(KT):
                for half in range(2):
                    nc.tensor.matmul(
                        pv_ps[:, half * CH : (half + 1) * CH],
                        lhsT=v_aug[:, t, :],
                        rhs=pTs[t][:, half * CH : (half + 1) * CH],
                        start=(t == 0), stop=(t == KT - 1),
                    )
            pvT = pvT_pool.tile([NF + 1, CP], BF16)
            nc.vector.tensor_copy(pvT[:, :CH], pv_ps[: NF + 1, :CH])
            nc.vector.tensor_copy(pvT[:, CH:], pv_ps[: NF + 1, CH:])

            # transpose back to [128 q, 65], normalize, store.
            # pvT col j = half*512 + i*128 + p  <->  q = 8p + 2i + half
            out_sb = out_pool.tile([P, 4, 2, NF], F32)
            for r in range(2):
                ot_ps = ot_ps_pool.tile([P, 4, NF + 2], BF16)
                rec = rec_pool.tile([P, 4], F32)
                for s in range(4):
                    j = 4 * r + s
                    nc.tensor.transpose(
                        ot_ps[:, s, : NF + 1],
                        pvT[:, j * P : (j + 1) * P],
                        identb[: NF + 1, : NF + 1],
                    )
                nc.vector.reciprocal(rec, ot_ps[:, :, NF])
                # single batched normalize: out = ot * rec (broadcast over f)
                nc.vector.tensor_tensor(
                    out_sb[:, :, r, :],
                    ot_ps[:, :, :NF],
                    rec.unsqueeze(2).broadcast_to([P, 4, NF]),
                    mybir.AluOpType.mult,
                )
            # out_sb[p, s, r, :] = out[q] with q = c*1024 + 8p + 2s + r,
            # so flattening (s r) gives 8 consecutive rows per partition
            nc.sync.dma_start(
                out=y[c * CP : (c + 1) * CP, :].rearrange(
                    "(p j) f -> p j f", p=P
                ),
                in_=out_sb.rearrange("p s r f -> p (s r) f"),
            )

    nc.compile()
    return nc


def get_nc():
    if "nc" not in _CACHE:
        _CACHE["nc"] = _build_nc()
    return _CACHE["nc"]


def run(inputs: dict, trace: bool = False):
    """Run on the 8 NeuronCores. Returns (out [8,8192,64], exec_time_ns)."""
    from concourse.bass_utils import run_bass_kernel_spmd

    nc = get_nc()
    in_maps = [
        {
            "x": np.ascontiguousarray(inputs["x"][b]),
            "kv": np.ascontiguousarray(inputs["kv"][b]),
            "Wq": np.asarray(inputs["Wq"]),
            "Wk": np.asarray(inputs["Wk"]),
            "Wv": np.asarray(inputs["Wv"]),
        }
        for b in range(B)
    ]
    res = run_bass_kernel_spmd(
        nc, in_maps, core_ids=list(range(B)), trace=trace
    )
    out = np.stack([res.results[b]["y"] for b in range(B)])
    return out, res.exec_time_ns


def kernel(**inputs) -> np.ndarray:
    out, _ = run(inputs, trace=False)
    return out


# revision 9
# speedup vs baseline: 1.0811x; 1.0096x over previous
"""Trainium2 Bass kernel for per-batch (block-diagonal) attention.

Computes, for each batch b independently:
    q = x[b] @ Wq ; k = kv[b] @ Wk ; v = kv[b] @ Wv
    out[b] = softmax(q @ k^T) @ v

Sharding: data-parallel over B=8 across the 8 NeuronCores (one batch
element per core). Each core holds the full 64x64 weights.

Math used on-device (per core, x:[8192,64], kv:[1024,64]):
    A   = Wq @ Wk^T            (64x64, fp32)
    U^T = A  @ kv^T            (64x1024, f16)
    S^T = U  @ x^T             -> scores^T tiles [128k, 512q] (fp16 in, fp32 acc)
    P^T = exp(S^T)             (ACT, PSUM->SBUF, bf16 out)
    outT_aug = [v | 1 | 0pad]^T @ P^T  (bf16, PSUM fp32 accumulate;
                                        row 64 = softmax denominator)
    out = outT_aug[0:64].T / denom   (PE transpose back + DVE recip * mul)

Layout trick: SBUF tiles hold 8 *consecutive* HBM rows per partition
(row n = 8p + j), so every big DMA (x in, kv in, y out) moves 2KB
contiguous per partition (128 descriptors instead of 512) - ~4x faster
transfers and lower latency.  This permutes key order (tile t = keys
congruent t mod 8) and query order (q = 8p + 2i + half); key order
cancels identically through U^T/P^T/v_aug, and the y store view
inverts the query permutation.

Engine streams are FIFO in emission order, so the prologue emits each
engine's ops in expected-data-arrival order: sync queue carries
kv(first half) / x-chunk0(half A) / kv(second half); scalar queue
carries wq, wk, x-chunk0(half B), wv.  x casts are emitted on vector
BEFORE the kv-dependent DVE ops so they are not queued behind them.

The kernel is ACT(exp)-bound in steady state: 8.39M exps/core at
128 lanes * 1.2 GHz ~= 55 us floor, +~300cyc/ACTIVATE overhead.
"""

from contextlib import ExitStack

import numpy as np

import concourse.mybir as mybir
from concourse import bacc
from concourse.masks import make_identity
from concourse.tile import TileContext

B, LQ, LK, NF = 8, 8192, 1024, 64
P = 128
CH = 512             # queries per PSUM-bank-sized slice
KT = LK // P         # 8 key tiles
F32 = mybir.dt.float32
F16 = mybir.dt.float16
BF16 = mybir.dt.bfloat16
EXP = mybir.ActivationFunctionType.Exp

_CACHE: dict = {}


def _build_nc():
    nc = bacc.Bacc("TRN2", target_bir_lowering=False, debug=False)
    x = nc.dram_tensor("x", [LQ, NF], F32, kind="ExternalInput").ap()
    kv = nc.dram_tensor("kv", [LK, NF], F32, kind="ExternalInput").ap()
    wq = nc.dram_tensor("Wq", [NF, NF], F32, kind="ExternalInput").ap()
    wk = nc.dram_tensor("Wk", [NF, NF], F32, kind="ExternalInput").ap()
    wv = nc.dram_tensor("Wv", [NF, NF], F32, kind="ExternalInput").ap()
    y = nc.dram_tensor("y", [LQ, NF], F32, kind="ExternalOutput").ap()

    CP = 2 * CH  # 1024 queries per chunk-pair

    with TileContext(nc) as tc, ExitStack() as ctx:
        singles = ctx.enter_context(tc.tile_pool(name="singles", bufs=1))
        xin = ctx.enter_context(tc.tile_pool(name="xin", bufs=4))
        x16_pool = ctx.enter_context(tc.tile_pool(name="x16", bufs=4))

        # preload the exp table set ASAP so the ~2.7us load overlaps prologue
        warm = singles.tile([P, 1], F32)
        nc.vector.memset(warm, 0.0)
        nc.scalar.activation(out=warm, in_=warm, func=EXP)

        # ---- DMA submits first ----
        x_sb0 = xin.tile([P, 8, NF], F32)
        kv_sb = singles.tile([P, 8, NF], F32)
        wq_sb = singles.tile([NF, NF], F32)
        wk_sb = singles.tile([NF, NF], F32)
        wv_sb = singles.tile([NF, NF], F32)

        xv0 = x[0:CP, :].rearrange("(p j) f -> p j f", p=P)
        kv_v = kv.rearrange("(p r) f -> p r f", p=P)
        nc.sync.dma_start(out=kv_sb[:, :4, :], in_=kv_v[:, :4, :])
        nc.sync.dma_start(out=x_sb0[:, :4], in_=xv0[:, :4])
        nc.sync.dma_start(out=kv_sb[:, 4:, :], in_=kv_v[:, 4:, :])
        nc.scalar.dma_start(out=wq_sb, in_=wq)
        nc.scalar.dma_start(out=wk_sb, in_=wk)
        nc.scalar.dma_start(out=x_sb0[:, 4:], in_=xv0[:, 4:])
        nc.scalar.dma_start(out=wv_sb, in_=wv)

        # identities; gpsimd stream continues with kv casts below
        ident = singles.tile([P, P], F32)
        make_identity(nc, ident)
        ident16 = singles.tile([P, P], F16)
        nc.gpsimd.tensor_copy(ident16, ident)

        # ---- prologue compute: W^T, A^T(f16), kv^T(f16), U^T(f16) ----
        with tc.tile_pool(name="pro_ps", bufs=2, space="PSUM") as pro_ps:
            # PE stream: wqT, wkT, A^T first (weights land earliest)
            wqT = singles.tile([NF, NF], F32)
            wkT = singles.tile([NF, NF], F32)
            for w_sb, wT in ((wq_sb, wqT), (wk_sb, wkT)):
                t_ps = pro_ps.tile([NF, NF], F32, tag="pro")
                nc.tensor.transpose(t_ps, w_sb, ident[:NF, :NF])
                nc.vector.tensor_copy(wT, t_ps)
            at_ps = pro_ps.tile([NF, NF], F32, tag="pro")
            nc.tensor.matmul(at_ps, lhsT=wkT, rhs=wqT, start=True, stop=True)
            aT16 = singles.tile([NF, NF], F16)
            nc.vector.tensor_copy(aT16, at_ps)

            # x chunk-0 casts EARLY in the vector stream (before kv DVE work)
            x16_0 = x16_pool.tile([P, 8, NF], F16)
            nc.vector.tensor_copy(x16_0[:, :4], x_sb0[:, :4])
            nc.vector.tensor_copy(x16_0[:, 4:], x_sb0[:, 4:])

            # kv -> f16 -> kv^T [64, 1024] f16, piecewise: kt0-1 first.
            # key-tile t holds keys {8j + t}; order is consistent across
            # U^T, P^T and v_aug so the softmax result is unchanged.
            kv16 = singles.tile([P, 8, NF], F16)
            kvT16 = singles.tile([NF, LK], F16)
            uT = singles.tile([P, LK], F16)

            def kv_piece(i0, i1):
                nc.gpsimd.tensor_copy(
                    kv16[:, 2 * i0 : 2 * i1, :], kv_sb[:, 2 * i0 : 2 * i1, :]
                )
                for i in range(i0, i1):
                    kt_ps = pro_ps.tile([P, P], F16, tag="prokv")
                    nc.tensor.transpose(
                        kt_ps, kv16[:, 2 * i : 2 * i + 2, :], ident16
                    )
                    nc.vector.tensor_copy(
                        kvT16[:, (2 * i) * P : (2 * i + 1) * P], kt_ps[:NF, :]
                    )
                    nc.vector.tensor_copy(
                        kvT16[:, (2 * i + 1) * P : (2 * i + 2) * P], kt_ps[NF:, :]
                    )
                # U^T piece: f16 MM; cast twice from PSUM (partitions 0:64
                # and the 64:128 duplicate for the packed row-group MMs)
                c0, c1 = (2 * i0) * P, (2 * i1) * P
                for d0 in range(c0, c1, CH):
                    d1 = min(d0 + CH, c1)
                    ut_ps = pro_ps.tile([NF, CH], F32, tag="prou")
                    nc.tensor.matmul(
                        ut_ps[:, : d1 - d0], lhsT=aT16, rhs=kvT16[:, d0:d1],
                        start=True, stop=True,
                    )
                    nc.vector.tensor_copy(uT[:NF, d0:d1], ut_ps[:, : d1 - d0])
                    nc.vector.tensor_copy(uT[NF:, d0:d1], ut_ps[:, : d1 - d0])

            kv_piece(0, 1)   # kt0-1: fast path for the first score panels
            kv_piece(1, 2)   # kt2-3
            identb = singles.tile([P, P], BF16)
            nc.gpsimd.tensor_copy(identb, ident)
            kv_piece(2, 4)   # kt4-7
            wv16 = singles.tile([NF, NF], F16)
            nc.gpsimd.tensor_copy(wv16, wv_sb)

            # v_aug is filled later (inside chunk-pair 0, after its scores are
            # queued) so the first exp doesn't wait behind the v matmuls
            v_aug = singles.tile([P, KT, P], BF16)
            ones_sb = singles.tile([P, 1], F32)
            nc.vector.memset(ones_sb, 1.0)

        # ---- main loop over query chunk-pairs (1024 queries each) ----
        xT_pool = ctx.enter_context(tc.tile_pool(name="xT", bufs=3))
        pT_pool = ctx.enter_context(tc.tile_pool(name="pT", bufs=12))
        pvT_pool = ctx.enter_context(tc.tile_pool(name="pvT", bufs=3))
        out_pool = ctx.enter_context(tc.tile_pool(name="outsb", bufs=3))
        rec_pool = ctx.enter_context(tc.tile_pool(name="rec", bufs=4))

        xt_ps_pool = ctx.enter_context(
            tc.tile_pool(name="xt_ps", bufs=1, space="PSUM")
        )
        ot_ps_pool = ctx.enter_context(
            tc.tile_pool(name="ot_ps", bufs=1, space="PSUM")
        )
        sc_ps_pool = ctx.enter_context(
            tc.tile_pool(name="sc_ps", bufs=2, space="PSUM")
        )
        pv_ps_pool = ctx.enter_context(
            tc.tile_pool(name="pv_ps", bufs=1, space="PSUM")
        )

        for c in range(LQ // CP):
            if c == 0:
                x_sb = x_sb0
                x16 = x16_0
            else:
                x_sb = xin.tile([P, 8, NF], F32)
                nc.sync.dma_start(
                    out=x_sb,
                    in_=x[c * CP : (c + 1) * CP, :].rearrange(
                        "(p j) f -> p j f", p=P
                    ),
                )
                x16 = x16_pool.tile([P, 8, NF], F16)
                nc.gpsimd.tensor_copy(x16, x_sb)
            # stacked transpose: xt partitions 0:64 = even-j queries'
            # features, 64:128 = odd-j queries'
            xt_ps = xt_ps_pool.tile([P, 4, P], F16, tag="xt")
            for i in range(4):
                nc.tensor.transpose(
                    xt_ps[:, i, :], x16[:, 2 * i : 2 * i + 2, :], ident16
                )
            xTc = xT_pool.tile([P, 4, P], F16)
            nc.vector.tensor_copy(xTc, xt_ps)

            # scores^T: per key tile, 2 row-group-packed MMs (even/odd qs)
            # -> exp -> P^T [128, KT, 1024] (bf16)
            pTs = []
            for t in range(KT):
                s_ps = sc_ps_pool.tile([P, CP], F32)
                nc.tensor.matmul(
                    s_ps[:, :CH],
                    lhsT=uT[:NF, t * P : (t + 1) * P],
                    rhs=xTc[:NF],
                    start=True, stop=True,
                    tile_position=(0, 0),
                )
                nc.tensor.matmul(
                    s_ps[:, CH:],
                    lhsT=uT[NF:, t * P : (t + 1) * P],
                    rhs=xTc[NF:],
                    start=True, stop=True,
                    tile_position=(64, 0),
                )
                pT_t = pT_pool.tile([P, CP], BF16, tag="pT")
                pTs.append(pT_t)
                nc.scalar.activation(out=pT_t, in_=s_ps, func=EXP)

            if c == 0:
                # fill v_aug now: [v | 1 | 0pad] per key tile, bf16
                for t in range(KT):
                    v_ps = xt_ps_pool.tile([P, NF], F32, tag="xt")
                    nc.tensor.matmul(
                        v_ps, lhsT=kvT16[:, t * P : (t + 1) * P], rhs=wv16,
                        start=True, stop=True,
                    )
                    nc.vector.tensor_copy(v_aug[:, t, :NF], v_ps)
                    nc.vector.tensor_copy(v_aug[:, t, NF : NF + 1], ones_sb)
                    nc.vector.memset(v_aug[:, t, NF + 1 :], 0.0)

            # PV: outT_aug [128, 1024] accumulated over key tiles (row 64 = denom)
            pv_ps = pv_ps_pool.tile([P, CP], F32)
            for t in range(KT):
                for half in range(2):
                    nc.tensor.matmul(
                        pv_ps[:, half * CH : (half + 1) * CH],
                        lhsT=v_aug[:, t, :],
                        rhs=pTs[t][:, half * CH : (half + 1) * CH],
                        start=(t == 0), stop=(t == KT - 1),
                    )
            pvT = pvT_pool.tile([NF + 1, CP], BF16)
            nc.vector.tensor_copy(pvT[:, :CH], pv_ps[: NF + 1, :CH])
            nc.vector.tensor_copy(pvT[:, CH:], pv_ps[: NF + 1, CH:])

            # transpose back to [128 q, 65], normalize, store.
            # pvT col j = half*512 + i*128 + p  <->  q = 8p + 2i + half
            out_sb = out_pool.tile([P, 4, 2, NF], F32)
            for r in range(2):
                ot_ps = ot_ps_pool.tile([P, 4, NF + 2], BF16)
                rec = rec_pool.tile([P, 4], F32)
                for s in range(4):
                    j = 4 * r + s
                    nc.tensor.transpose(
                        ot_ps[:, s, : NF + 1],
                        pvT[:, j * P : (j + 1) * P],
                        identb[: NF + 1, : NF + 1],
                    )
                nc.vector.reciprocal(rec, ot_ps[:, :, NF])
                # single batched normalize: out = ot * rec (broadcast over f)
                nc.vector.tensor_tensor(
                    out_sb[:, :, r, :],
                    ot_ps[:, :, :NF],
                    rec.unsqueeze(2).broadcast_to([P, 4, NF]),
                    mybir.AluOpType.mult,
                )
            # out_sb[p, s, r, :] = out[q] with q = c*1024 + 8p + 2s + r,
            # so flattening (s r) gives 8 consecutive rows per partition
            nc.sync.dma_start(
                out=y[c * CP : (c + 1) * CP, :].rearrange(
                    "(p j) f -> p j f", p=P
                ),
                in_=out_sb.rearrange("p s r f -> p (s r) f"),
            )

    nc.compile()
    return nc


def get_nc():
    if "nc" not in _CACHE:
        _CACHE["nc"] = _build_nc()
    return _CACHE["nc"]


def run(inputs: dict, trace: bool = False):
    """Run on the 8 NeuronCores. Returns (out [8,8192,64], exec_time_ns)."""
    from concourse.bass_utils import run_bass_kernel_spmd

    nc = get_nc()
    in_maps = [
        {
            "x": np.ascontiguousarray(inputs["x"][b]),
            "kv": np.ascontiguousarray(inputs["kv"][b]),
            "Wq": np.asarray(inputs["Wq"]),
            "Wk": np.asarray(inputs["Wk"]),
            "Wv": np.asarray(inputs["Wv"]),
        }
        for b in range(B)
    ]
    res = run_bass_kernel_spmd(
        nc, in_maps, core_ids=list(range(B)), trace=trace
    )
    out = np.stack([res.results[b]["y"] for b in range(B)])
    return out, res.exec_time_ns


def kernel(**inputs) -> np.ndarray:
    out, _ = run(inputs, trace=False)
    return out


# revision 15
# speedup vs baseline: 1.0983x; 1.0159x over previous
"""Trainium2 Bass kernel for per-batch (block-diagonal) attention.

Computes, for each batch b independently:
    q = x[b] @ Wq ; k = kv[b] @ Wk ; v = kv[b] @ Wv
    out[b] = softmax(q @ k^T) @ v

Sharding: data-parallel over B=8 across the 8 NeuronCores (one batch
element per core). Each core holds the full 64x64 weights.

Math used on-device (per core, x:[8192,64], kv:[1024,64]):
    A   = Wq @ Wk^T            (64x64, fp32)
    U^T = A  @ kv^T            (64x1024, f16)
    S^T = U  @ x^T             -> scores^T tiles [128k, 512q] (fp16 in, fp32 acc)
    P^T = exp(S^T)             (ACT, PSUM->SBUF, bf16 out)
    outT_aug = [v | 1 | 0pad]^T @ P^T  (bf16, PSUM fp32 accumulate;
                                        row 64 = softmax denominator)
    out = outT_aug[0:64].T / denom   (PE transpose back + DVE recip * mul)

Layout trick: SBUF tiles hold 8 *consecutive* HBM rows per partition
(row n = 8p + j), so every big DMA (x in, kv in, y out) moves 2KB
contiguous per partition (128 descriptors instead of 512) - ~4x faster
transfers and lower latency.  This permutes key order (tile t = keys
congruent t mod 8) and query order (q = 8p + 2i + half); key order
cancels identically through U^T/P^T/v_aug, and the y store view
inverts the query permutation.

Engine streams are FIFO in emission order, so the prologue emits each
engine's ops in expected-data-arrival order: sync queue carries
kv(first half) / x-chunk0(half A) / kv(second half); scalar queue
carries wq, wk, x-chunk0(half B), wv.  x casts are emitted on vector
BEFORE the kv-dependent DVE ops so they are not queued behind them.

The kernel is ACT(exp)-bound in steady state: 8.39M exps/core at
128 lanes * 1.2 GHz ~= 55 us floor, +~300cyc/ACTIVATE overhead.
"""

from contextlib import ExitStack

import numpy as np

import concourse.mybir as mybir
from concourse import bacc
from concourse.masks import make_identity
from concourse.tile import TileContext

B, LQ, LK, NF = 8, 8192, 1024, 64
P = 128
CH = 512             # queries per PSUM-bank-sized slice
KT = LK // P         # 8 key tiles
F32 = mybir.dt.float32
F16 = mybir.dt.float16
BF16 = mybir.dt.bfloat16
EXP = mybir.ActivationFunctionType.Exp

_CACHE: dict = {}


def _build_nc():
    nc = bacc.Bacc("TRN2", target_bir_lowering=False, debug=False)
    x = nc.dram_tensor("x", [LQ, NF], F32, kind="ExternalInput").ap()
    kv = nc.dram_tensor("kv", [LK, NF], F32, kind="ExternalInput").ap()
    wq = nc.dram_tensor("Wq", [NF, NF], F32, kind="ExternalInput").ap()
    wk = nc.dram_tensor("Wk", [NF, NF], F32, kind="ExternalInput").ap()
    wv = nc.dram_tensor("Wv", [NF, NF], F32, kind="ExternalInput").ap()
    y = nc.dram_tensor("y", [LQ, NF], F32, kind="ExternalOutput").ap()

    CP = 2 * CH  # 1024 queries per chunk-pair

    with TileContext(nc) as tc, ExitStack() as ctx:
        singles = ctx.enter_context(tc.tile_pool(name="singles", bufs=1))
        # bufs=2: only one x prefetch DMA can be in flight during the
        # prologue, so it can't round-robin-steal much ring bandwidth from
        # the critical kv + x-chunk0 transfers
        xin = ctx.enter_context(tc.tile_pool(name="xin", bufs=2))
        x16_pool = ctx.enter_context(tc.tile_pool(name="x16", bufs=4))

        # preload the exp table set ASAP so the ~2.7us load overlaps prologue
        warm = singles.tile([P, 1], F32)
        nc.vector.memset(warm, 0.0)
        nc.scalar.activation(out=warm, in_=warm, func=EXP)

        # ---- DMA submits first ----
        x_sb0 = xin.tile([P, 8, NF], F32)
        kv_sb = singles.tile([P, 8, NF], F32)
        wq_sb = singles.tile([NF, NF], F32)
        wk_sb = singles.tile([NF, NF], F32)
        wv_sb = singles.tile([NF, NF], F32)

        # DMA descriptors round-robin across all transfers queued in a ring
        # at ~fixed cost per descriptor, so the critical kv + x-chunk0 get
        # the sync ring to themselves (128 x 2KB descriptors each); the
        # small weights and the steady-state x prefetches use the scalar
        # ring so they never crowd the critical path.
        xv0 = x[0:CP, :].rearrange("(p j) f -> p j f", p=P)
        kv_v = kv.rearrange("(p r) f -> p r f", p=P)
        nc.sync.dma_start(out=x_sb0, in_=xv0)
        nc.sync.dma_start(out=kv_sb, in_=kv_v)
        nc.scalar.dma_start(out=wq_sb, in_=wq)
        nc.scalar.dma_start(out=wk_sb, in_=wk)
        nc.scalar.dma_start(out=wv_sb, in_=wv)

        # identities; gpsimd stream continues with kv casts below
        ident = singles.tile([P, P], F32)
        make_identity(nc, ident)
        ident16 = singles.tile([P, P], F16)
        nc.gpsimd.tensor_copy(ident16, ident)

        # ---- prologue compute: W^T, A^T(f16), kv^T(f16), U^T(f16) ----
        with tc.tile_pool(name="pro_ps", bufs=2, space="PSUM") as pro_ps:
            # PE stream: wqT, wkT, A^T first (weights land earliest)
            wqT = singles.tile([NF, NF], F32)
            wkT = singles.tile([NF, NF], F32)
            for w_sb, wT in ((wq_sb, wqT), (wk_sb, wkT)):
                t_ps = pro_ps.tile([NF, NF], F32, tag="pro")
                nc.tensor.transpose(t_ps, w_sb, ident[:NF, :NF])
                nc.vector.tensor_copy(wT, t_ps)
            at_ps = pro_ps.tile([NF, NF], F32, tag="pro")
            nc.tensor.matmul(at_ps, lhsT=wkT, rhs=wqT, start=True, stop=True)
            aT16 = singles.tile([NF, NF], F16)
            nc.vector.tensor_copy(aT16, at_ps)

            # x chunk-0 casts EARLY in the vector stream (before kv DVE work)
            x16_0 = x16_pool.tile([P, 8, NF], F16)
            nc.vector.tensor_copy(x16_0[:, :4], x_sb0[:, :4])
            nc.vector.tensor_copy(x16_0[:, 4:], x_sb0[:, 4:])

            # kv -> f16 -> kv^T [64, 1024] f16, piecewise: kt0-1 first.
            # key-tile t holds keys {8j + t}; order is consistent across
            # U^T, P^T and v_aug so the softmax result is unchanged.
            kv16 = singles.tile([P, 8, NF], F16)
            kvT16 = singles.tile([NF, LK], F16)
            uT = singles.tile([P, LK], F16)

            def kv_piece(i0, i1):
                nc.gpsimd.tensor_copy(
                    kv16[:, 2 * i0 : 2 * i1, :], kv_sb[:, 2 * i0 : 2 * i1, :]
                )
                for i in range(i0, i1):
                    kt_ps = pro_ps.tile([P, P], F16, tag="prokv")
                    nc.tensor.transpose(
                        kt_ps, kv16[:, 2 * i : 2 * i + 2, :], ident16
                    )
                    nc.vector.tensor_copy(
                        kvT16[:, (2 * i) * P : (2 * i + 1) * P], kt_ps[:NF, :]
                    )
                    nc.vector.tensor_copy(
                        kvT16[:, (2 * i + 1) * P : (2 * i + 2) * P], kt_ps[NF:, :]
                    )
                # U^T piece: f16 MM; cast twice from PSUM (partitions 0:64
                # and the 64:128 duplicate for the packed row-group MMs)
                c0, c1 = (2 * i0) * P, (2 * i1) * P
                for d0 in range(c0, c1, CH):
                    d1 = min(d0 + CH, c1)
                    ut_ps = pro_ps.tile([NF, CH], F32, tag="prou")
                    nc.tensor.matmul(
                        ut_ps[:, : d1 - d0], lhsT=aT16, rhs=kvT16[:, d0:d1],
                        start=True, stop=True,
                    )
                    nc.vector.tensor_copy(uT[:NF, d0:d1], ut_ps[:, : d1 - d0])
                    nc.vector.tensor_copy(uT[NF:, d0:d1], ut_ps[:, : d1 - d0])

            kv_piece(0, 1)   # kt0-1: fast path for the first score panels
            kv_piece(1, 2)   # kt2-3
            identb = singles.tile([P, P], BF16)
            nc.gpsimd.tensor_copy(identb, ident)
            kv_piece(2, 4)   # kt4-7
            wv16 = singles.tile([NF, NF], F16)
            nc.gpsimd.tensor_copy(wv16, wv_sb)

            # v_aug is filled later (inside chunk-pair 0, after its scores are
            # queued) so the first exp doesn't wait behind the v matmuls
            v_aug = singles.tile([P, KT, P], BF16)
            ones_sb = singles.tile([P, 1], F32)
            nc.vector.memset(ones_sb, 1.0)

        # ---- main loop over query chunk-pairs (1024 queries each) ----
        xT_pool = ctx.enter_context(tc.tile_pool(name="xT", bufs=3))
        pT_pool = ctx.enter_context(tc.tile_pool(name="pT", bufs=12))
        pvT_pool = ctx.enter_context(tc.tile_pool(name="pvT", bufs=3))
        out_pool = ctx.enter_context(tc.tile_pool(name="outsb", bufs=3))
        rec_pool = ctx.enter_context(tc.tile_pool(name="rec", bufs=4))

        xt_ps_pool = ctx.enter_context(
            tc.tile_pool(name="xt_ps", bufs=1, space="PSUM")
        )
        ot_ps_pool = ctx.enter_context(
            tc.tile_pool(name="ot_ps", bufs=1, space="PSUM")
        )
        sc_ps_pool = ctx.enter_context(
            tc.tile_pool(name="sc_ps", bufs=2, space="PSUM")
        )
        pv_ps_pool = ctx.enter_context(
            tc.tile_pool(name="pv_ps", bufs=1, space="PSUM")
        )

        for c in range(LQ // CP):
            if c == 0:
                x_sb = x_sb0
                x16 = x16_0
            else:
                x_sb = xin.tile([P, 8, NF], F32)
                nc.sync.dma_start(
                    out=x_sb,
                    in_=x[c * CP : (c + 1) * CP, :].rearrange(
                        "(p j) f -> p j f", p=P
                    ),
                )
                x16 = x16_pool.tile([P, 8, NF], F16)
                nc.gpsimd.tensor_copy(x16, x_sb)
            # stacked transpose: xt partitions 0:64 = even-j queries'
            # features, 64:128 = odd-j queries'
            xt_ps = xt_ps_pool.tile([P, 4, P], F16, tag="xt")
            for i in range(4):
                nc.tensor.transpose(
                    xt_ps[:, i, :], x16[:, 2 * i : 2 * i + 2, :], ident16
                )
            xTc = xT_pool.tile([P, 4, P], F16)
            nc.vector.tensor_copy(xTc, xt_ps)

            # scores^T: per key tile, 2 row-group-packed MMs (even/odd qs)
            # -> exp -> P^T [128, KT, 1024] (bf16)
            pTs = []
            for t in range(KT):
                s_ps = sc_ps_pool.tile([P, CP], F32)
                nc.tensor.matmul(
                    s_ps[:, :CH],
                    lhsT=uT[:NF, t * P : (t + 1) * P],
                    rhs=xTc[:NF],
                    start=True, stop=True,
                    tile_position=(0, 0),
                )
                nc.tensor.matmul(
                    s_ps[:, CH:],
                    lhsT=uT[NF:, t * P : (t + 1) * P],
                    rhs=xTc[NF:],
                    start=True, stop=True,
                    tile_position=(64, 0),
                )
                pT_t = pT_pool.tile([P, CP], BF16, tag="pT")
                pTs.append(pT_t)
                nc.scalar.activation(out=pT_t, in_=s_ps, func=EXP)

            if c == 0:
                # fill v_aug now: [v | 1 | 0pad] per key tile, bf16
                for t in range(KT):
                    v_ps = xt_ps_pool.tile([P, NF], F32, tag="xt")
                    nc.tensor.matmul(
                        v_ps, lhsT=kvT16[:, t * P : (t + 1) * P], rhs=wv16,
                        start=True, stop=True,
                    )
                    nc.vector.tensor_copy(v_aug[:, t, :NF], v_ps)
                    nc.vector.tensor_copy(v_aug[:, t, NF : NF + 1], ones_sb)
                    nc.vector.memset(v_aug[:, t, NF + 1 :], 0.0)

            # PV: outT_aug [128, 1024] accumulated over key tiles (row 64 = denom)
            pv_ps = pv_ps_pool.tile([P, CP], F32)
            for t in range(KT):
                for half in range(2):
                    nc.tensor.matmul(
                        pv_ps[:, half * CH : (half + 1) * CH],
                        lhsT=v_aug[:, t, :],
                        rhs=pTs[t][:, half * CH : (half + 1) * CH],
                        start=(t == 0), stop=(t == KT - 1),
                    )
            pvT = pvT_pool.tile([NF + 1, CP], BF16)
            nc.vector.tensor_copy(pvT[:, :CH], pv_ps[: NF + 1, :CH])
            nc.vector.tensor_copy(pvT[:, CH:], pv_ps[: NF + 1, CH:])

            # transpose back to [128 q, 65], normalize, store.
            # pvT col j = half*512 + i*128 + p  <->  q = 8p + 2i + half
            out_sb = out_pool.tile([P, 4, 2, NF], F32)
            for r in range(2):
                ot_ps = ot_ps_pool.tile([P, 4, NF + 2], BF16)
                rec = rec_pool.tile([P, 4], F32)
                for s in range(4):
                    j = 4 * r + s
                    nc.tensor.transpose(
                        ot_ps[:, s, : NF + 1],
                        pvT[:, j * P : (j + 1) * P],
                        identb[: NF + 1, : NF + 1],
                    )
                nc.vector.reciprocal(rec, ot_ps[:, :, NF])
                # single batched normalize: out = ot * rec (broadcast over f)
                nc.vector.tensor_tensor(
                    out_sb[:, :, r, :],
                    ot_ps[:, :, :NF],
                    rec.unsqueeze(2).broadcast_to([P, 4, NF]),
                    mybir.AluOpType.mult,
                )
            # out_sb[p, s, r, :] = out[q] with q = c*1024 + 8p + 2s + r,
            # so flattening (s r) gives 8 consecutive rows per partition
            nc.sync.dma_start(
                out=y[c * CP : (c + 1) * CP, :].rearrange(
                    "(p j) f -> p j f", p=P
                ),
                in_=out_sb.rearrange("p s r f -> p (s r) f"),
            )

    nc.compile()
    return nc


def get_nc():
    if "nc" not in _CACHE:
        _CACHE["nc"] = _build_nc()
    return _CACHE["nc"]


def run(inputs: dict, trace: bool = False):
    """Run on the 8 NeuronCores. Returns (out [8,8192,64], exec_time_ns)."""
    from concourse.bass_utils import run_bass_kernel_spmd

    nc = get_nc()
    in_maps = [
        {
            "x": np.ascontiguousarray(inputs["x"][b]),
            "kv": np.ascontiguousarray(inputs["kv"][b]),
            "Wq": np.asarray(inputs["Wq"]),
            "Wk": np.asarray(inputs["Wk"]),
            "Wv": np.asarray(inputs["Wv"]),
        }
        for b in range(B)
    ]
    res = run_bass_kernel_spmd(
        nc, in_maps, core_ids=list(range(B)), trace=trace
    )
    out = np.stack([res.results[b]["y"] for b in range(B)])
    return out, res.exec_time_ns


def kernel(**inputs) -> np.ndarray:
    out, _ = run(inputs, trace=False)
    return out


# revision 17
# speedup vs baseline: 1.0993x; 1.0009x over previous
"""Trainium2 Bass kernel for per-batch (block-diagonal) attention.

Computes, for each batch b independently:
    q = x[b] @ Wq ; k = kv[b] @ Wk ; v = kv[b] @ Wv
    out[b] = softmax(q @ k^T) @ v

Sharding: data-parallel over B=8 across the 8 NeuronCores (one batch
element per core). Each core holds the full 64x64 weights.

Math used on-device (per core, x:[8192,64], kv:[1024,64]):
    A   = Wq @ Wk^T            (64x64, fp32)
    U^T = A  @ kv^T            (64x1024, f16)
    S^T = U  @ x^T             -> scores^T tiles [128k, 512q] (fp16 in, fp32 acc)
    P^T = exp(S^T)             (ACT, PSUM->SBUF, bf16 out)
    outT_aug = [v | 1 | 0pad]^T @ P^T  (bf16, PSUM fp32 accumulate;
                                        row 64 = softmax denominator)
    out = outT_aug[0:64].T / denom   (PE transpose back + DVE recip * mul)

Layout trick: SBUF tiles hold 8 *consecutive* HBM rows per partition
(row n = 8p + j), so every big DMA (x in, kv in, y out) moves 2KB
contiguous per partition (128 descriptors instead of 512) - ~4x faster
transfers and lower latency.  This permutes key order (tile t = keys
congruent t mod 8) and query order (q = 8p + 2i + half); key order
cancels identically through U^T/P^T/v_aug, and the y store view
inverts the query permutation.

Engine streams are FIFO in emission order, so the prologue emits each
engine's ops in expected-data-arrival order: sync queue carries
kv(first half) / x-chunk0(half A) / kv(second half); scalar queue
carries wq, wk, x-chunk0(half B), wv.  x casts are emitted on vector
BEFORE the kv-dependent DVE ops so they are not queued behind them.

The kernel is ACT(exp)-bound in steady state: 8.39M exps/core at
128 lanes * 1.2 GHz ~= 55 us floor, +~300cyc/ACTIVATE overhead.
"""

from contextlib import ExitStack

import numpy as np

import concourse.mybir as mybir
from concourse import bacc
from concourse.masks import make_identity
from concourse.tile import TileContext

B, LQ, LK, NF = 8, 8192, 1024, 64
P = 128
CH = 512             # queries per PSUM-bank-sized slice
KT = LK // P         # 8 key tiles
F32 = mybir.dt.float32
F16 = mybir.dt.float16
BF16 = mybir.dt.bfloat16
EXP = mybir.ActivationFunctionType.Exp

_CACHE: dict = {}


def _build_nc():
    nc = bacc.Bacc("TRN2", target_bir_lowering=False, debug=False)
    x = nc.dram_tensor("x", [LQ, NF], F32, kind="ExternalInput").ap()
    kv = nc.dram_tensor("kv", [LK, NF], F32, kind="ExternalInput").ap()
    wq = nc.dram_tensor("Wq", [NF, NF], F32, kind="ExternalInput").ap()
    wk = nc.dram_tensor("Wk", [NF, NF], F32, kind="ExternalInput").ap()
    wv = nc.dram_tensor("Wv", [NF, NF], F32, kind="ExternalInput").ap()
    y = nc.dram_tensor("y", [LQ, NF], F32, kind="ExternalOutput").ap()

    CP = 2 * CH  # 1024 queries per chunk-pair

    with TileContext(nc) as tc, ExitStack() as ctx:
        singles = ctx.enter_context(tc.tile_pool(name="singles", bufs=1))
        # bufs=2: only one x prefetch DMA can be in flight during the
        # prologue, so it can't round-robin-steal much ring bandwidth from
        # the critical kv + x-chunk0 transfers
        xin = ctx.enter_context(tc.tile_pool(name="xin", bufs=2))
        x16_pool = ctx.enter_context(tc.tile_pool(name="x16", bufs=4))

        # preload the exp table set ASAP so the ~2.7us load overlaps prologue
        warm = singles.tile([P, 1], F32)
        nc.vector.memset(warm, 0.0)
        nc.scalar.activation(out=warm, in_=warm, func=EXP)

        # ---- DMA submits first ----
        x_sb0 = xin.tile([P, 8, NF], F32)
        kv_sb = singles.tile([P, 8, NF], F32)
        wq_sb = singles.tile([NF, NF], F32)
        wk_sb = singles.tile([NF, NF], F32)
        wv_sb = singles.tile([NF, NF], F32)

        # DMA descriptors round-robin across all transfers queued in a ring
        # at ~fixed cost per descriptor, so the critical kv + x-chunk0 get
        # the sync ring to themselves (128 x 2KB descriptors each); the
        # small weights and the steady-state x prefetches use the scalar
        # ring so they never crowd the critical path.
        xv0 = x[0:CP, :].rearrange("(p j) f -> p j f", p=P)
        kv_v = kv.rearrange("(p r) f -> p r f", p=P)
        nc.sync.dma_start(out=x_sb0, in_=xv0)
        nc.sync.dma_start(out=kv_sb, in_=kv_v)
        nc.scalar.dma_start(out=wq_sb, in_=wq)
        nc.scalar.dma_start(out=wk_sb, in_=wk)
        nc.scalar.dma_start(out=wv_sb, in_=wv)

        # identities; gpsimd stream continues with kv casts below
        ident = singles.tile([P, P], F32)
        make_identity(nc, ident)
        ident16 = singles.tile([P, P], F16)
        nc.gpsimd.tensor_copy(ident16, ident)

        # ---- prologue compute: W^T, A^T(f16), kv^T(f16), U^T(f16) ----
        with tc.tile_pool(name="pro_ps", bufs=2, space="PSUM") as pro_ps:
            # PE stream: wqT, wkT, A^T first (weights land earliest)
            wqT = singles.tile([NF, NF], F32)
            wkT = singles.tile([NF, NF], F32)
            for w_sb, wT in ((wq_sb, wqT), (wk_sb, wkT)):
                t_ps = pro_ps.tile([NF, NF], F32, tag="pro")
                nc.tensor.transpose(t_ps, w_sb, ident[:NF, :NF])
                nc.vector.tensor_copy(wT, t_ps)
            at_ps = pro_ps.tile([NF, NF], F32, tag="pro")
            nc.tensor.matmul(at_ps, lhsT=wkT, rhs=wqT, start=True, stop=True)
            aT16 = singles.tile([NF, NF], F16)
            nc.vector.tensor_copy(aT16, at_ps)

            # x chunk-0 casts EARLY in the vector stream (before kv DVE work)
            x16_0 = x16_pool.tile([P, 8, NF], F16)
            nc.vector.tensor_copy(x16_0[:, :4], x_sb0[:, :4])
            nc.vector.tensor_copy(x16_0[:, 4:], x_sb0[:, 4:])

            # kv -> f16 -> kv^T [64, 1024] f16, piecewise: kt0-1 first.
            # key-tile t holds keys {8j + t}; order is consistent across
            # U^T, P^T and v_aug so the softmax result is unchanged.
            kv16 = singles.tile([P, 8, NF], F16)
            kvT16 = singles.tile([NF, LK], F16)
            uT = singles.tile([P, LK], F16)

            def kv_piece(i0, i1):
                nc.gpsimd.tensor_copy(
                    kv16[:, 2 * i0 : 2 * i1, :], kv_sb[:, 2 * i0 : 2 * i1, :]
                )
                for i in range(i0, i1):
                    kt_ps = pro_ps.tile([P, P], F16, tag="prokv")
                    nc.tensor.transpose(
                        kt_ps, kv16[:, 2 * i : 2 * i + 2, :], ident16
                    )
                    nc.vector.tensor_copy(
                        kvT16[:, (2 * i) * P : (2 * i + 1) * P], kt_ps[:NF, :]
                    )
                    nc.vector.tensor_copy(
                        kvT16[:, (2 * i + 1) * P : (2 * i + 2) * P], kt_ps[NF:, :]
                    )
                # U^T piece: f16 MM; cast twice from PSUM (partitions 0:64
                # and the 64:128 duplicate for the packed row-group MMs)
                c0, c1 = (2 * i0) * P, (2 * i1) * P
                for d0 in range(c0, c1, CH):
                    d1 = min(d0 + CH, c1)
                    ut_ps = pro_ps.tile([NF, CH], F32, tag="prou")
                    nc.tensor.matmul(
                        ut_ps[:, : d1 - d0], lhsT=aT16, rhs=kvT16[:, d0:d1],
                        start=True, stop=True,
                    )
                    nc.vector.tensor_copy(uT[:NF, d0:d1], ut_ps[:, : d1 - d0])
                    nc.vector.tensor_copy(uT[NF:, d0:d1], ut_ps[:, : d1 - d0])

            kv_piece(0, 1)   # kt0-1: fast path for the first score panels
            kv_piece(1, 2)   # kt2-3
            identb = singles.tile([P, P], BF16)
            nc.gpsimd.tensor_copy(identb, ident)
            kv_piece(2, 4)   # kt4-7
            wv16 = singles.tile([NF, NF], F16)
            nc.gpsimd.tensor_copy(wv16, wv_sb)

            # chunk-0 x transpose INSIDE the prologue pool: the main loop's
            # xt_ps bank aliases a prologue bank, and waiting for that
            # bank's last prologue reader (the uT casts) would stall the
            # first scores by several us (PSUM WAR hazard).
            xt0_ps = pro_ps.tile([P, 4, P], F16, tag="xt0")
            for i in range(4):
                nc.tensor.transpose(
                    xt0_ps[:, i, :], x16_0[:, 2 * i : 2 * i + 2, :], ident16
                )
            xTc0 = singles.tile([P, 4, P], F16)
            nc.vector.tensor_copy(xTc0, xt0_ps)

            # v_aug is filled later (inside chunk-pair 0, after its scores are
            # queued) so the first exp doesn't wait behind the v matmuls
            v_aug = singles.tile([P, KT, P], BF16)
            ones_sb = singles.tile([P, 1], F32)
            nc.vector.memset(ones_sb, 1.0)

        # ---- main loop over query chunk-pairs (1024 queries each) ----
        xT_pool = ctx.enter_context(tc.tile_pool(name="xT", bufs=3))
        pT_pool = ctx.enter_context(tc.tile_pool(name="pT", bufs=12))
        pvT_pool = ctx.enter_context(tc.tile_pool(name="pvT", bufs=3))
        out_pool = ctx.enter_context(tc.tile_pool(name="outsb", bufs=3))
        rec_pool = ctx.enter_context(tc.tile_pool(name="rec", bufs=4))

        xt_ps_pool = ctx.enter_context(
            tc.tile_pool(name="xt_ps", bufs=1, space="PSUM")
        )
        ot_ps_pool = ctx.enter_context(
            tc.tile_pool(name="ot_ps", bufs=1, space="PSUM")
        )
        sc_ps_pool = ctx.enter_context(
            tc.tile_pool(name="sc_ps", bufs=2, space="PSUM")
        )
        pv_ps_pool = ctx.enter_context(
            tc.tile_pool(name="pv_ps", bufs=1, space="PSUM")
        )

        for c in range(LQ // CP):
            if c == 0:
                x_sb = x_sb0
                x16 = x16_0
            else:
                x_sb = xin.tile([P, 8, NF], F32)
                nc.sync.dma_start(
                    out=x_sb,
                    in_=x[c * CP : (c + 1) * CP, :].rearrange(
                        "(p j) f -> p j f", p=P
                    ),
                )
                x16 = x16_pool.tile([P, 8, NF], F16)
                nc.gpsimd.tensor_copy(x16, x_sb)
            # stacked transpose: xt partitions 0:64 = even-j queries'
            # features, 64:128 = odd-j queries' (chunk 0 was done in the
            # prologue to dodge a PSUM bank-reuse stall)
            if c == 0:
                xTc = xTc0
            else:
                xt_ps = xt_ps_pool.tile([P, 4, P], F16, tag="xt")
                for i in range(4):
                    nc.tensor.transpose(
                        xt_ps[:, i, :], x16[:, 2 * i : 2 * i + 2, :], ident16
                    )
                xTc = xT_pool.tile([P, 4, P], F16)
                nc.vector.tensor_copy(xTc, xt_ps)

            # scores^T: per key tile, 2 row-group-packed MMs (even/odd qs)
            # -> exp -> P^T [128, KT, 1024] (bf16)
            pTs = []
            for t in range(KT):
                s_ps = sc_ps_pool.tile([P, CP], F32)
                nc.tensor.matmul(
                    s_ps[:, :CH],
                    lhsT=uT[:NF, t * P : (t + 1) * P],
                    rhs=xTc[:NF],
                    start=True, stop=True,
                    tile_position=(0, 0),
                )
                nc.tensor.matmul(
                    s_ps[:, CH:],
                    lhsT=uT[NF:, t * P : (t + 1) * P],
                    rhs=xTc[NF:],
                    start=True, stop=True,
                    tile_position=(64, 0),
                )
                pT_t = pT_pool.tile([P, CP], BF16, tag="pT")
                pTs.append(pT_t)
                nc.scalar.activation(out=pT_t, in_=s_ps, func=EXP)

            if c == 0:
                # fill v_aug now: [v | 1 | 0pad] per key tile, bf16
                for t in range(KT):
                    v_ps = xt_ps_pool.tile([P, NF], F32, tag="xt")
                    nc.tensor.matmul(
                        v_ps, lhsT=kvT16[:, t * P : (t + 1) * P], rhs=wv16,
                        start=True, stop=True,
                    )
                    nc.vector.tensor_copy(v_aug[:, t, :NF], v_ps)
                    nc.vector.tensor_copy(v_aug[:, t, NF : NF + 1], ones_sb)
                    nc.vector.memset(v_aug[:, t, NF + 1 :], 0.0)

            # PV: outT_aug [128, 1024] accumulated over key tiles (row 64 = denom)
            pv_ps = pv_ps_pool.tile([P, CP], F32)
            for t in range(KT):
                for half in range(2):
                    nc.tensor.matmul(
                        pv_ps[:, half * CH : (half + 1) * CH],
                        lhsT=v_aug[:, t, :],
                        rhs=pTs[t][:, half * CH : (half + 1) * CH],
                        start=(t == 0), stop=(t == KT - 1),
                    )
            pvT = pvT_pool.tile([NF + 1, CP], BF16)
            nc.vector.tensor_copy(pvT[:, :CH], pv_ps[: NF + 1, :CH])
            nc.vector.tensor_copy(pvT[:, CH:], pv_ps[: NF + 1, CH:])

            # transpose back to [128 q, 65], normalize, store.
            # pvT col j = half*512 + i*128 + p  <->  q = 8p + 2i + half
            out_sb = out_pool.tile([P, 4, 2, NF], F32)
            for r in range(2):
                ot_ps = ot_ps_pool.tile([P, 4, NF + 2], BF16)
                rec = rec_pool.tile([P, 4], F32)
                for s in range(4):
                    j = 4 * r + s
                    nc.tensor.transpose(
                        ot_ps[:, s, : NF + 1],
                        pvT[:, j * P : (j + 1) * P],
                        identb[: NF + 1, : NF + 1],
                    )
                nc.vector.reciprocal(rec, ot_ps[:, :, NF])
                # single batched normalize: out = ot * rec (broadcast over f)
                nc.vector.tensor_tensor(
                    out_sb[:, :, r, :],
                    ot_ps[:, :, :NF],
                    rec.unsqueeze(2).broadcast_to([P, 4, NF]),
                    mybir.AluOpType.mult,
                )
            # out_sb[p, s, r, :] = out[q] with q = c*1024 + 8p + 2s + r,
            # so flattening (s r) gives 8 consecutive rows per partition
            nc.sync.dma_start(
                out=y[c * CP : (c + 1) * CP, :].rearrange(
                    "(p j) f -> p j f", p=P
                ),
                in_=out_sb.rearrange("p s r f -> p (s r) f"),
            )

    nc.compile()
    return nc


def get_nc():
    if "nc" not in _CACHE:
        _CACHE["nc"] = _build_nc()
    return _CACHE["nc"]


def run(inputs: dict, trace: bool = False):
    """Run on the 8 NeuronCores. Returns (out [8,8192,64], exec_time_ns)."""
    from concourse.bass_utils import run_bass_kernel_spmd

    nc = get_nc()
    in_maps = [
        {
            "x": np.ascontiguousarray(inputs["x"][b]),
            "kv": np.ascontiguousarray(inputs["kv"][b]),
            "Wq": np.asarray(inputs["Wq"]),
            "Wk": np.asarray(inputs["Wk"]),
            "Wv": np.asarray(inputs["Wv"]),
        }
        for b in range(B)
    ]
    res = run_bass_kernel_spmd(
        nc, in_maps, core_ids=list(range(B)), trace=trace
    )
    out = np.stack([res.results[b]["y"] for b in range(B)])
    return out, res.exec_time_ns


def kernel(**inputs) -> np.ndarray:
    out, _ = run(inputs, trace=False)
    return out


# revision 21
# speedup vs baseline: 1.1048x; 1.0049x over previous
"""Trainium2 Bass kernel for per-batch (block-diagonal) attention.

Computes, for each batch b independently:
    q = x[b] @ Wq ; k = kv[b] @ Wk ; v = kv[b] @ Wv
    out[b] = softmax(q @ k^T) @ v

Sharding: data-parallel over B=8 across the 8 NeuronCores (one batch
element per core). Each core holds the full 64x64 weights.

Math used on-device (per core, x:[8192,64], kv:[1024,64]):
    A   = Wq @ Wk^T            (64x64, fp32)
    U^T = A  @ kv^T            (64x1024, f16)
    S^T = U  @ x^T             -> scores^T tiles [128k, 512q] (fp16 in, fp32 acc)
    P^T = exp(S^T)             (ACT, PSUM->SBUF, bf16 out)
    outT_aug = [v | 1 | 0pad]^T @ P^T  (bf16, PSUM fp32 accumulate;
                                        row 64 = softmax denominator)
    out = outT_aug[0:64].T / denom   (PE transpose back + DVE recip * mul)

Layout trick: SBUF tiles hold 8 *consecutive* HBM rows per partition
(row n = 8p + j), so every big DMA (x in, kv in, y out) moves 2KB
contiguous per partition (128 descriptors instead of 512) - ~4x faster
transfers and lower latency.  This permutes key order (tile t = keys
congruent t mod 8) and query order (q = 8p + 2i + half); key order
cancels identically through U^T/P^T/v_aug, and the y store view
inverts the query permutation.

Engine streams are FIFO in emission order, so the prologue emits each
engine's ops in expected-data-arrival order: sync queue carries
kv(first half) / x-chunk0(half A) / kv(second half); scalar queue
carries wq, wk, x-chunk0(half B), wv.  x casts are emitted on vector
BEFORE the kv-dependent DVE ops so they are not queued behind them.

The kernel is ACT(exp)-bound in steady state: 8.39M exps/core at
128 lanes * 1.2 GHz ~= 55 us floor, +~300cyc/ACTIVATE overhead.
"""

from contextlib import ExitStack

import numpy as np

import concourse.mybir as mybir
from concourse import bacc
from concourse.masks import make_identity
from concourse.tile import TileContext

B, LQ, LK, NF = 8, 8192, 1024, 64
P = 128
CH = 512             # queries per PSUM-bank-sized slice
KT = LK // P         # 8 key tiles
F32 = mybir.dt.float32
F16 = mybir.dt.float16
BF16 = mybir.dt.bfloat16
EXP = mybir.ActivationFunctionType.Exp

_CACHE: dict = {}


def _build_nc():
    nc = bacc.Bacc("TRN2", target_bir_lowering=False, debug=False)
    x = nc.dram_tensor("x", [LQ, NF], F32, kind="ExternalInput").ap()
    kv = nc.dram_tensor("kv", [LK, NF], F32, kind="ExternalInput").ap()
    wq = nc.dram_tensor("Wq", [NF, NF], F32, kind="ExternalInput").ap()
    wk = nc.dram_tensor("Wk", [NF, NF], F32, kind="ExternalInput").ap()
    wv = nc.dram_tensor("Wv", [NF, NF], F32, kind="ExternalInput").ap()
    y = nc.dram_tensor("y", [LQ, NF], F32, kind="ExternalOutput").ap()

    CP = 2 * CH  # 1024 queries per chunk-pair

    with TileContext(nc) as tc, ExitStack() as ctx:
        singles = ctx.enter_context(tc.tile_pool(name="singles", bufs=1))
        # bufs=2: only one x prefetch DMA can be in flight during the
        # prologue, so it can't round-robin-steal much ring bandwidth from
        # the critical kv + x-chunk0 transfers
        xin = ctx.enter_context(tc.tile_pool(name="xin", bufs=2))
        x16_pool = ctx.enter_context(tc.tile_pool(name="x16", bufs=4))

        # preload the exp table set ASAP so the ~2.7us load overlaps prologue
        warm = singles.tile([P, 1], F32)
        nc.vector.memset(warm, 0.0)
        nc.scalar.activation(out=warm, in_=warm, func=EXP)

        # ---- DMA submits first ----
        x_sb0 = xin.tile([P, 8, NF], F32)
        kv_sb = singles.tile([P, 8, NF], F32)
        wq_sb = singles.tile([NF, NF], F32)
        wk_sb = singles.tile([NF, NF], F32)
        wv_sb = singles.tile([NF, NF], F32)

        # DMA descriptors round-robin across all transfers queued in a ring
        # at ~fixed cost per descriptor, so the critical kv + x-chunk0 get
        # the sync ring to themselves (128 x 2KB descriptors each); the
        # small weights and the steady-state x prefetches use the scalar
        # ring so they never crowd the critical path.
        xv0 = x[0:CP, :].rearrange("(p j) f -> p j f", p=P)
        kv_v = kv.rearrange("(p r) f -> p r f", p=P)
        nc.sync.dma_start(out=x_sb0, in_=xv0)
        nc.sync.dma_start(out=kv_sb, in_=kv_v)
        nc.scalar.dma_start(out=wq_sb, in_=wq)
        nc.scalar.dma_start(out=wk_sb, in_=wk)
        nc.scalar.dma_start(out=wv_sb, in_=wv)

        pT_pool = ctx.enter_context(tc.tile_pool(name="pT", bufs=12))
        sc_ps_pool = ctx.enter_context(
            tc.tile_pool(name="sc_ps", bufs=2, space="PSUM")
        )

        # identities; gpsimd stream continues with kv casts below
        ident = singles.tile([P, P], F32)
        make_identity(nc, ident)
        ident16 = singles.tile([P, P], F16)
        nc.gpsimd.tensor_copy(ident16, ident)

        # ---- prologue compute: W^T, A^T(f16), kv^T(f16), U^T(f16) ----
        with tc.tile_pool(name="pro_ps", bufs=1, space="PSUM") as pro_ps:
            # PE stream: wqT, wkT, A^T first (weights land earliest)
            wqT = singles.tile([NF, NF], F32)
            wkT = singles.tile([NF, NF], F32)
            for w_sb, wT in ((wq_sb, wqT), (wk_sb, wkT)):
                t_ps = pro_ps.tile([NF, NF], F32, tag="pro")
                nc.tensor.transpose(t_ps, w_sb, ident[:NF, :NF])
                nc.vector.tensor_copy(wT, t_ps)
            at_ps = pro_ps.tile([NF, NF], F32, tag="pro")
            nc.tensor.matmul(at_ps, lhsT=wkT, rhs=wqT, start=True, stop=True)
            aT16 = singles.tile([NF, NF], F16)
            nc.vector.tensor_copy(aT16, at_ps)

            # x chunk-0 casts EARLY in the vector stream (before kv DVE work)
            x16_0 = x16_pool.tile([P, 8, NF], F16)
            nc.vector.tensor_copy(x16_0[:, :4], x_sb0[:, :4])
            nc.vector.tensor_copy(x16_0[:, 4:], x_sb0[:, 4:])

            # kv -> f16 -> kv^T [64, 1024] f16, piecewise: kt0-1 first.
            # key-tile t holds keys {8j + t}; order is consistent across
            # U^T, P^T and v_aug so the softmax result is unchanged.
            kv16 = singles.tile([P, 8, NF], F16)
            kvT16 = singles.tile([NF, LK], F16)
            uT = singles.tile([P, LK], F16)

            def kv_piece(i0, i1):
                nc.gpsimd.tensor_copy(
                    kv16[:, 2 * i0 : 2 * i1, :], kv_sb[:, 2 * i0 : 2 * i1, :]
                )
                for i in range(i0, i1):
                    kt_ps = pro_ps.tile([P, P], F16, tag="prokv")
                    nc.tensor.transpose(
                        kt_ps, kv16[:, 2 * i : 2 * i + 2, :], ident16
                    )
                    nc.vector.tensor_copy(
                        kvT16[:, (2 * i) * P : (2 * i + 1) * P], kt_ps[:NF, :]
                    )
                    nc.vector.tensor_copy(
                        kvT16[:, (2 * i + 1) * P : (2 * i + 2) * P], kt_ps[NF:, :]
                    )
                # U^T piece: f16 MM; cast twice from PSUM (partitions 0:64
                # and the 64:128 duplicate for the packed row-group MMs)
                c0, c1 = (2 * i0) * P, (2 * i1) * P
                for d0 in range(c0, c1, CH):
                    d1 = min(d0 + CH, c1)
                    ut_ps = pro_ps.tile([NF, CH], F32, tag="prou")
                    nc.tensor.matmul(
                        ut_ps[:, : d1 - d0], lhsT=aT16, rhs=kvT16[:, d0:d1],
                        start=True, stop=True,
                    )
                    nc.vector.tensor_copy(uT[:NF, d0:d1], ut_ps[:, : d1 - d0])
                    nc.vector.tensor_copy(uT[NF:, d0:d1], ut_ps[:, : d1 - d0])

            kv_piece(0, 1)   # kt0-1: fast path for the first score panels

            # chunk-0 x transpose INSIDE the prologue pool: the main loop's
            # xt_ps bank aliases a prologue bank, and waiting for that
            # bank's last prologue reader (the uT casts) would stall the
            # first scores by several us (PSUM WAR hazard).
            xt0_ps = pro_ps.tile([P, 4, P], F16, tag="xt0")
            for i in range(4):
                nc.tensor.transpose(
                    xt0_ps[:, i, :], x16_0[:, 2 * i : 2 * i + 2, :], ident16
                )
            xTc0 = singles.tile([P, 4, P], F16)
            nc.vector.tensor_copy(xTc0, xt0_ps)

            # first two score panels emitted HERE so the PE stream reaches
            # them right after uT[kt0-1] - not queued behind the kt2-7 U^T
            # matmuls (engines execute their streams strictly in order)
            pTs0 = []
            for t in range(2):
                s_ps = sc_ps_pool.tile([P, CP], F32)
                nc.tensor.matmul(
                    s_ps[:, :CH],
                    lhsT=uT[:NF, t * P : (t + 1) * P],
                    rhs=xTc0[:NF],
                    start=True, stop=True,
                    tile_position=(0, 0),
                )
                nc.tensor.matmul(
                    s_ps[:, CH:],
                    lhsT=uT[NF:, t * P : (t + 1) * P],
                    rhs=xTc0[NF:],
                    start=True, stop=True,
                    tile_position=(64, 0),
                )
                pT_t = pT_pool.tile([P, CP], BF16, tag="pT")
                pTs0.append(pT_t)
                nc.scalar.activation(out=pT_t, in_=s_ps, func=EXP)

            kv_piece(1, 2)   # kt2-3
            identb = singles.tile([P, P], BF16)
            nc.gpsimd.tensor_copy(identb, ident)
            kv_piece(2, 4)   # kt4-7
            wv16 = singles.tile([NF, NF], F16)
            nc.gpsimd.tensor_copy(wv16, wv_sb)

            # v_aug is filled later (inside chunk-pair 0, after its scores are
            # queued) so the first exp doesn't wait behind the v matmuls
            v_aug = singles.tile([P, KT, P], BF16)
            ones_sb = singles.tile([P, 1], F32)
            nc.vector.memset(ones_sb, 1.0)

        # ---- main loop over query chunk-pairs (1024 queries each) ----
        xT_pool = ctx.enter_context(tc.tile_pool(name="xT", bufs=3))
        pvT_pool = ctx.enter_context(tc.tile_pool(name="pvT", bufs=3))
        out_pool = ctx.enter_context(tc.tile_pool(name="outsb", bufs=3))
        rec_pool = ctx.enter_context(tc.tile_pool(name="rec", bufs=4))

        xt_ps_pool = ctx.enter_context(
            tc.tile_pool(name="xt_ps", bufs=1, space="PSUM")
        )
        ot_ps_pool = ctx.enter_context(
            tc.tile_pool(name="ot_ps", bufs=1, space="PSUM")
        )
        pv_ps_pool = ctx.enter_context(
            tc.tile_pool(name="pv_ps", bufs=1, space="PSUM")
        )

        for c in range(LQ // CP):
            if c == 0:
                x_sb = x_sb0
                x16 = x16_0
            else:
                x_sb = xin.tile([P, 8, NF], F32)
                nc.sync.dma_start(
                    out=x_sb,
                    in_=x[c * CP : (c + 1) * CP, :].rearrange(
                        "(p j) f -> p j f", p=P
                    ),
                )
                x16 = x16_pool.tile([P, 8, NF], F16)
                nc.gpsimd.tensor_copy(x16, x_sb)
            # stacked transpose: xt partitions 0:64 = even-j queries'
            # features, 64:128 = odd-j queries' (chunk 0 was done in the
            # prologue to dodge a PSUM bank-reuse stall)
            if c == 0:
                xTc = xTc0
            else:
                xt_ps = xt_ps_pool.tile([P, 4, P], F16, tag="xt")
                for i in range(4):
                    nc.tensor.transpose(
                        xt_ps[:, i, :], x16[:, 2 * i : 2 * i + 2, :], ident16
                    )
                xTc = xT_pool.tile([P, 4, P], F16)
                nc.vector.tensor_copy(xTc, xt_ps)

            # scores^T: per key tile, 2 row-group-packed MMs (even/odd qs)
            # -> exp -> P^T [128, KT, 1024] (bf16)
            pTs = list(pTs0) if c == 0 else []
            for t in range(2 if c == 0 else 0, KT):
                s_ps = sc_ps_pool.tile([P, CP], F32)
                nc.tensor.matmul(
                    s_ps[:, :CH],
                    lhsT=uT[:NF, t * P : (t + 1) * P],
                    rhs=xTc[:NF],
                    start=True, stop=True,
                    tile_position=(0, 0),
                )
                nc.tensor.matmul(
                    s_ps[:, CH:],
                    lhsT=uT[NF:, t * P : (t + 1) * P],
                    rhs=xTc[NF:],
                    start=True, stop=True,
                    tile_position=(64, 0),
                )
                pT_t = pT_pool.tile([P, CP], BF16, tag="pT")
                pTs.append(pT_t)
                nc.scalar.activation(out=pT_t, in_=s_ps, func=EXP)

            if c == 0:
                # fill v_aug now: [v | 1 | 0pad] per key tile, bf16
                for t in range(KT):
                    v_ps = xt_ps_pool.tile([P, NF], F32, tag="xt")
                    nc.tensor.matmul(
                        v_ps, lhsT=kvT16[:, t * P : (t + 1) * P], rhs=wv16,
                        start=True, stop=True,
                    )
                    nc.vector.tensor_copy(v_aug[:, t, :NF], v_ps)
                    nc.vector.tensor_copy(v_aug[:, t, NF : NF + 1], ones_sb)
                    nc.vector.memset(v_aug[:, t, NF + 1 :], 0.0)

            # PV: outT_aug [128, 1024] accumulated over key tiles (row 64 = denom)
            pv_ps = pv_ps_pool.tile([P, CP], F32)
            for t in range(KT):
                for half in range(2):
                    nc.tensor.matmul(
                        pv_ps[:, half * CH : (half + 1) * CH],
                        lhsT=v_aug[:, t, :],
                        rhs=pTs[t][:, half * CH : (half + 1) * CH],
                        start=(t == 0), stop=(t == KT - 1),
                    )
            pvT = pvT_pool.tile([NF + 1, CP], BF16)
            nc.vector.tensor_copy(pvT[:, :CH], pv_ps[: NF + 1, :CH])
            nc.vector.tensor_copy(pvT[:, CH:], pv_ps[: NF + 1, CH:])

            # transpose back to [128 q, 65], normalize, store.
            # pvT col j = half*512 + i*128 + p  <->  q = 8p + 2i + half
            out_sb = out_pool.tile([P, 4, 2, NF], F32)
            for r in range(2):
                ot_ps = ot_ps_pool.tile([P, 4, NF + 2], BF16)
                rec = rec_pool.tile([P, 4], F32)
                for s in range(4):
                    j = 4 * r + s
                    nc.tensor.transpose(
                        ot_ps[:, s, : NF + 1],
                        pvT[:, j * P : (j + 1) * P],
                        identb[: NF + 1, : NF + 1],
                    )
                nc.vector.reciprocal(rec, ot_ps[:, :, NF])
                # single batched normalize: out = ot * rec (broadcast over f)
                nc.vector.tensor_tensor(
                    out_sb[:, :, r, :],
                    ot_ps[:, :, :NF],
                    rec.unsqueeze(2).broadcast_to([P, 4, NF]),
                    mybir.AluOpType.mult,
                )
            # out_sb[p, s, r, :] = out[q] with q = c*1024 + 8p + 2s + r,
            # so flattening (s r) gives 8 consecutive rows per partition
            nc.sync.dma_start(
                out=y[c * CP : (c + 1) * CP, :].rearrange(
                    "(p j) f -> p j f", p=P
                ),
                in_=out_sb.rearrange("p s r f -> p (s r) f"),
            )

    nc.compile()
    return nc


def get_nc():
    if "nc" not in _CACHE:
        _CACHE["nc"] = _build_nc()
    return _CACHE["nc"]


def run(inputs: dict, trace: bool = False):
    """Run on the 8 NeuronCores. Returns (out [8,8192,64], exec_time_ns)."""
    from concourse.bass_utils import run_bass_kernel_spmd

    nc = get_nc()
    in_maps = [
        {
            "x": np.ascontiguousarray(inputs["x"][b]),
            "kv": np.ascontiguousarray(inputs["kv"][b]),
            "Wq": np.asarray(inputs["Wq"]),
            "Wk": np.asarray(inputs["Wk"]),
            "Wv": np.asarray(inputs["Wv"]),
        }
        for b in range(B)
    ]
    res = run_bass_kernel_spmd(
        nc, in_maps, core_ids=list(range(B)), trace=trace
    )
    out = np.stack([res.results[b]["y"] for b in range(B)])
    return out, res.exec_time_ns


def kernel(**inputs) -> np.ndarray:
    out, _ = run(inputs, trace=False)
    return out
